# revision 1
# baseline (speedup 1.0000x reference)
"""Distributed Trainium2 kernel for GQA attention block (B=2,T=1024,D=2560,Nq=32,Nkv=8,H=128).

Sharding: 8 cores = 2 batches x 4 head-groups. Core c handles batch c//4 and
q-heads [8g:8g+8), kv-heads [2g:2g+2) where g=c%4. Attention is head-local;
two 8-core AllToAlls (even token chunks / odd token chunks) re-shard
heads->tokens before o_proj. Shard s of each A2A carries this core's heads for
token chunk 2*(s%4)(+1); each core keeps same-batch rows via a data-driven
select. Each core emits a [256, 2560] token-slice of the final output.
"""

import ml_dtypes
import numpy as np

import concourse.bass as bass
import concourse.mybir as mybir
import concourse.tile as tile
from concourse import bacc
from concourse.bass_utils import run_bass_kernel_spmd

F32 = mybir.dt.float32
F32R = mybir.dt.float32r
BF16 = mybir.dt.bfloat16

B, T, D, NQ, NKV, H = 2, 1024, 2560, 32, 8, 128
NDT = D // 128  # 20 contraction tiles
NTC = T // 128  # 8 token chunks
ROPE_THETA = 1000000.0
MROPE_SECTION = (24, 20, 20)
NORM_EPS = 1e-6
SOFT_SCALE = H ** -0.5
NEG = -1e30

EXP_ACT = mybir.ActivationFunctionType.Exp
SQUARE_ACT = mybir.ActivationFunctionType.Square
SQRT_ACT = mybir.ActivationFunctionType.Sqrt
MUL_OP = mybir.AluOpType.mult
ADD_OP = mybir.AluOpType.add

_LAST = None


def _norm_rope(nc, smpool, spool, ps, nh, eps_t, ct, s0t, s1t, tagp):
    """RMS-norm over h + rope for nh heads sitting in psum ps[:, :nh*128]."""
    w = nh * 128
    qn = spool.tile([128, w], F32, tag=f"qn{tagp}")
    ssq = smpool.tile([128, nh], F32, tag="ssq")
    for hh in range(nh):
        sq = smpool.tile([128, 128], F32, tag="sq")
        nc.scalar.activation(sq[:, :], ps[:, hh * 128:(hh + 1) * 128], SQUARE_ACT,
                             accum_out=ssq[:, hh:hh + 1])
    srt = smpool.tile([128, nh], F32, tag="srt")
    nc.scalar.activation(srt[:, :], ssq[:, :], SQRT_ACT, bias=eps_t[:, :], scale=1.0 / H)
    rsq = smpool.tile([128, nh], F32, tag="rsq")
    nc.vector.reciprocal(rsq[:, :], srt[:, :])
    for hh in range(nh):
        sl = slice(hh * 128, (hh + 1) * 128)
        nc.vector.tensor_scalar_mul(qn[:, sl], ps[:, sl], rsq[:, hh:hh + 1])
    qm = spool.tile([128, w], F32, tag=f"qm{tagp}")
    qr = spool.tile([128, w], F32, tag=f"qr{tagp}")
    cb = ct.unsqueeze(1).broadcast_to([128, nh, 128])
    s0b = s0t.unsqueeze(1).broadcast_to([128, nh, 64])
    s1b = s1t.unsqueeze(1).broadcast_to([128, nh, 64])
    qn4 = qn[:, :].rearrange("p (h x) -> p h x", h=nh)
    qm4 = qm[:, :].rearrange("p (h x) -> p h x", h=nh)
    qr4 = qr[:, :].rearrange("p (h x) -> p h x", h=nh)
    nc.vector.tensor_mul(qm4, qn4, cb)
    t1 = spool.tile([128, nh * 64], F32, tag=f"t1{tagp}")
    t14 = t1[:, :].rearrange("p (h x) -> p h x", h=nh)
    nc.vector.tensor_mul(t14, qn4[:, :, 64:128], s0b)
    nc.vector.tensor_sub(qr4[:, :, 0:64], qm4[:, :, 0:64], t14)
    t2 = spool.tile([128, nh * 64], F32, tag=f"t2{tagp}")
    t24 = t2[:, :].rearrange("p (h x) -> p h x", h=nh)
    nc.vector.tensor_mul(t24, qn4[:, :, 0:64], s1b)
    nc.vector.tensor_add(qr4[:, :, 64:128], qm4[:, :, 64:128], t24)
    return qr


def _build_nc():
    nc = bacc.Bacc(None, target_bir_lowering=False, num_devices=8)

    xt_e = nc.declare_dram_parameter("xt", [NTC, NDT, 128, 128], F32R, isOutput=False)
    wq0_e = nc.declare_dram_parameter("wq0", [NDT, 128, 512], F32R, isOutput=False)
    wq1_e = nc.declare_dram_parameter("wq1", [NDT, 128, 512], F32R, isOutput=False)
    wkv_e = nc.declare_dram_parameter("wkv", [NDT, 128, 512], F32R, isOutput=False)
    cq_e = nc.declare_dram_parameter("cq", [T, 128], F32, isOutput=False)
    s0q_e = nc.declare_dram_parameter("s0q", [T, 64], F32, isOutput=False)
    s1q_e = nc.declare_dram_parameter("s1q", [T, 64], F32, isOutput=False)
    ck_e = nc.declare_dram_parameter("ck", [T, 128], F32, isOutput=False)
    s0k_e = nc.declare_dram_parameter("s0k", [T, 64], F32, isOutput=False)
    s1k_e = nc.declare_dram_parameter("s1k", [T, 64], F32, isOutput=False)
    maska_e = nc.declare_dram_parameter("maska", [NTC, 128, 128], F32, isOutput=False)
    identf_e = nc.declare_dram_parameter("identf", [128, 128], F32, isOutput=False)
    identb_e = nc.declare_dram_parameter("identb", [128, 128], BF16, isOutput=False)
    wob_e = nc.declare_dram_parameter("wob", [128, NQ, D], BF16, isOutput=False)
    bsel_e = nc.declare_dram_parameter("bsel", [128, 1], F32, isOutput=False)
    bsm_e = nc.declare_dram_parameter("bsm", [128, 1], F32, isOutput=False)
    out_e = nc.declare_dram_parameter("out", [256, D], F32, isOutput=True)

    with tile.TileContext(nc) as tc:
        with (
            tc.tile_pool(name="const", bufs=1) as cpool,
            tc.tile_pool(name="dram", bufs=1, space="DRAM") as dpool,
        ):
            eps_t = cpool.tile([128, 1], F32, tag="eps")
            nc.gpsimd.memset(eps_t[:, :], NORM_EPS)
            identb = cpool.tile([128, 128], BF16, tag="identb")
            nc.sync.dma_start(out=identb[:, :], in_=identb_e[:, :])

            a2aA_in = dpool.tile([8, 8, 128, 128], BF16, tag="a2aA_in")
            a2aA_out = dpool.tile([8, 8, 128, 128], BF16, tag="a2aA_out")
            a2aB_in = dpool.tile([8, 8, 128, 128], BF16, tag="a2aB_in")
            a2aB_out = dpool.tile([8, 8, 128, 128], BF16, tag="a2aB_out")

            with (
                tc.tile_pool(name="acts", bufs=1) as apool,
                tc.tile_pool(name="p3a", bufs=1) as a3pool,
            ):
                qT = apool.tile([128, 8, T], F32R, tag="qT")       # [h, qhead, t]
                kT = apool.tile([128, 2, T], F32R, tag="kT")       # [h, kvhead, t]
                vN = apool.tile([128, NTC, 256], BF16, tag="vN")   # [t_in_chunk, chunk, kv*128+h]

                # ====== Phase 1 ======
                with (
                    tc.tile_pool(name="p1tab", bufs=1) as tabpool,
                    tc.tile_pool(name="p1x", bufs=3) as xpool,
                    tc.tile_pool(name="p1w", bufs=24) as wpool,
                    tc.tile_pool(name="p1s", bufs=2) as spool,
                    tc.tile_pool(name="p1sm", bufs=6) as smpool,
                    tc.tile_pool(name="p1ps", bufs=2, space="PSUM") as pspool,
                    tc.tile_pool(name="p1pt", bufs=2, space="PSUM") as ptpool,
                ):
                    # first weight group + first x chunk go FIRST on the
                    # sync queue so the PE can start ASAP; tables after.
                    wts0 = []
                    for d in range(NDT):
                        wt = wpool.tile([128, 512], F32R, tag="wt")
                        nc.sync.dma_start(out=wt[:, :], in_=wq0_e[d, :, :])
                        wts0.append(wt)
                    xc0 = xpool.tile([128, NDT, 128], F32R, tag="xc")
                    nc.sync.dma_start(out=xc0[:, :, :],
                                      in_=xt_e[0, :, :, :].rearrange("d p t -> p d t"))

                    def ld(name, shp, src, rearr):
                        t = tabpool.tile(shp, F32, tag=name)
                        nc.sync.dma_start(out=t[:, :, :], in_=src[:, :].rearrange(rearr, p=128))
                        return t

                    cq = ld("cq", [128, NTC, 128], cq_e, "(c p) m -> p c m")
                    s0q = ld("s0q", [128, NTC, 64], s0q_e, "(c p) m -> p c m")
                    s1q = ld("s1q", [128, NTC, 64], s1q_e, "(c p) m -> p c m")
                    ck = ld("ck", [128, NTC, 128], ck_e, "(c p) m -> p c m")
                    s0k = ld("s0k", [128, NTC, 64], s0k_e, "(c p) m -> p c m")
                    s1k = ld("s1k", [128, NTC, 64], s1k_e, "(c p) m -> p c m")
                    maskt = apool.tile([128, NTC, 128], F32, tag="maskt")
                    nc.sync.dma_start(out=maskt[:, :, :], in_=maska_e[:, :, :].rearrange("i p m -> p i m"))
                    identf = apool.tile([128, 128], F32, tag="identf")
                    nc.sync.dma_start(out=identf[:, :], in_=identf_e[:, :])

                    # deferred PE-transposes: run one iteration behind the
                    # matmuls so the PE never waits on the DVE rope chain
                    pend1 = []

                    def flush1():
                        for qr_, tch_, heads_, dest in pend1:
                            for idx, head in enumerate(heads_):
                                pt = ptpool.tile([128, 128], F32, tag="pt")
                                nc.tensor.transpose(pt[:, :], qr_[:, idx * 128:(idx + 1) * 128], identf[:, :])
                                nc.vector.tensor_copy(dest[:, head, tch_ * 128:(tch_ + 1) * 128], pt[:, :])
                        pend1.clear()

                    for grp in range(3):
                        if grp == 0:
                            wts = wts0
                        else:
                            wdram = [wq0_e, wq1_e, wkv_e][grp]
                            wts = []
                            for d in range(NDT):
                                wt = wpool.tile([128, 512], F32R, tag="wt")
                                nc.sync.dma_start(out=wt[:, :], in_=wdram[d, :, :])
                                wts.append(wt)
                        for tch in range(NTC):
                            if grp == 0 and tch == 0:
                                xc = xc0
                            else:
                                xc = xpool.tile([128, NDT, 128], F32R, tag="xc")
                                nc.sync.dma_start(
                                    out=xc[:, :, :],
                                    in_=xt_e[tch, :, :, :].rearrange("d p t -> p d t"),
                                )
                            ps = pspool.tile([128, 512], F32, tag="ps")
                            for d in range(NDT):
                                nc.tensor.matmul(
                                    ps[:, :], xc[:, d, :], wts[d][:, :],
                                    start=(d == 0), stop=(d == NDT - 1),
                                )
                            flush1()
                            if grp < 2:
                                qr = _norm_rope(
                                    nc, smpool, spool, ps[:, :], 4, eps_t,
                                    cq[:, tch, :], s0q[:, tch, :], s1q[:, tch, :], "q")
                                pend1.append((qr, tch, [grp * 4 + hh for hh in range(4)], qT))
                            else:
                                kr = _norm_rope(
                                    nc, smpool, spool, ps[:, 0:256], 2, eps_t,
                                    ck[:, tch, :], s0k[:, tch, :], s1k[:, tch, :], "k")
                                pend1.append((kr, tch, [0, 1], kT))
                                nc.vector.tensor_copy(vN[:, tch, :], ps[:, 256:512])
                    flush1()

                # ====== Phase 2: attention (software-pipelined, evens then odds) ======
                def selects(par, a2a_out_t):
                    res = []
                    for g4 in range(4):
                        lo = lhpool.tile([128, 8, 128], BF16, tag="lo")
                        nc.gpsimd.dma_start(
                            out=lo[:, :, :],
                            in_=a2a_out_t[g4, :, :, :].rearrange("l h t -> h l t"),
                        )
                        hi = lhpool.tile([128, 8, 128], BF16, tag="hi")
                        nc.gpsimd.dma_start(
                            out=hi[:, :, :],
                            in_=a2a_out_t[4 + g4, :, :, :].rearrange("l h t -> h l t"),
                        )
                        tt = t3pool.tile([128, 8, 128], BF16, tag="tt")
                        nc.vector.tensor_scalar_mul(tt[:, :, :], lo[:, :, :], bsel[:, :])
                        aT = a3pool.tile([128, 8, 128], BF16, tag=f"aT{par}{g4}")
                        nc.vector.scalar_tensor_tensor(
                            aT[:, :, :], hi[:, :, :], bsm[:, :], tt[:, :, :],
                            op0=MUL_OP, op1=ADD_OP,
                        )
                        res.append(aT)
                    return res

                bsel = a3pool.tile([128, 1], F32, tag="bsel")
                nc.sync.dma_start(out=bsel[:, :], in_=bsel_e[:, :])
                bsm = a3pool.tile([128, 1], F32, tag="bsm")
                nc.sync.dma_start(out=bsm[:, :], in_=bsm_e[:, :])
                wo_ts = []
                aTe = []

                with (
                    tc.tile_pool(name="p3lh", bufs=4) as lhpool,
                    tc.tile_pool(name="p3t", bufs=2) as t3pool,
                    tc.tile_pool(name="p3w", bufs=3) as w3pool,
                    tc.tile_pool(name="p3o", bufs=3) as o3pool,
                    tc.tile_pool(name="p2a", bufs=3) as aapool,
                    tc.tile_pool(name="p2t", bufs=4) as tpool,
                    tc.tile_pool(name="p2d", bufs=3) as dpool2,
                    tc.tile_pool(name="p2o", bufs=2) as opool,
                    tc.tile_pool(name="p2sm", bufs=6) as sm2pool,
                    tc.tile_pool(name="p2sc", bufs=2, space="PSUM") as scpool,
                    tc.tile_pool(name="p2tr", bufs=2, space="PSUM") as trpool,
                    tc.tile_pool(name="p2av", bufs=2, space="PSUM") as avpool,
                ):
                    def finish(st):
                        hq, i, kv, at, drcp, oti = st
                        ov = avpool.tile([128, 128], F32, tag="ov")
                        for j in range(i + 1):
                            pt = trpool.tile([128, 128], F32, tag="ptr")
                            # transpose + softmax normalization in one PE op:
                            # regular matmul at_block.T @ diag(1/rowsum)
                            nc.tensor.matmul(pt[:, :], at[:, j * 128:(j + 1) * 128], drcp[:, :],
                                             start=True, stop=True)
                            atj = tpool.tile([128, 128], BF16, tag="atj")
                            nc.vector.tensor_copy(atj[:, :], pt[:, :])
                            nc.tensor.matmul(
                                ov[:, :],
                                vN[:, j, kv * 128:(kv + 1) * 128],
                                atj[:, :],
                                start=(j == 0),
                                stop=(j == i),
                            )
                        nc.vector.tensor_copy(oti[:, hq, :], ov[:, :])
                        if hq == 7:
                            r = i // 2
                            dst = a2aA_in if i % 2 == 0 else a2aB_in
                            nc.sync.dma_start(
                                out=dst[r, :, :, :].rearrange("l p t -> p l t"),
                                in_=oti[:, :, :],
                            )
                            nc.sync.dma_start(
                                out=dst[4 + r, :, :, :].rearrange("l p t -> p l t"),
                                in_=oti[:, :, :],
                            )

                    prev = None
                    for i in [0, 2, 4, 6, 1, 3, 5, 7]:
                        klen = 128 * (i + 1)
                        oti = opool.tile([128, 8, 128], BF16, tag="oti")
                        for hq in range(8):
                            kv = hq // 4
                            sc = scpool.tile([128, T], F32, tag="sc")
                            lhsT = qT[:, hq, i * 128:(i + 1) * 128]
                            for j0 in range(0, klen, 512):
                                j1 = min(klen, j0 + 512)
                                nc.tensor.matmul(
                                    sc[:, j0:j1], lhsT, kT[:, kv, j0:j1],
                                    start=True, stop=True,
                                )
                            nc.vector.tensor_add(sc[:, klen - 128:klen], sc[:, klen - 128:klen], maskt[:, i, :])
                            at = aapool.tile([128, T], BF16, tag="at")
                            rs = sm2pool.tile([128, 1], F32, tag="rs")
                            nc.scalar.activation(at[:, :klen], sc[:, :klen], EXP_ACT, scale=SOFT_SCALE, accum_out=rs[:, :])
                            rcp = sm2pool.tile([128, 1], F32, tag="rcp")
                            nc.vector.reciprocal(rcp[:, :], rs[:, :])
                            drcp = dpool2.tile([128, 128], BF16, tag="drcp")
                            nc.vector.tensor_scalar_mul(drcp[:, :], identb[:, :], rcp[:, :])
                            cur = (hq, i, kv, at, drcp, oti)
                            if prev is not None:
                                finish(prev)
                            prev = cur
                        if i == 6:
                            finish(prev)
                            prev = None
                            nc.gpsimd.collective_compute(
                                "AllToAll",
                                mybir.AluOpType.bypass,
                                replica_groups=[[0, 1, 2, 3, 4, 5, 6, 7]],
                                ins=[a2aA_in[:, :, :, :].opt()],
                                outs=[a2aA_out[:, :, :, :].opt()],
                            )
                            # prefetch the first o_proj weight chunks while
                            # the odd chunks compute (keep HBM pressure low
                            # so A2A#A is not starved)
                            for dch in range(2):
                                wo_t = w3pool.tile([128, NQ, 512], BF16, tag="wo_t")
                                nc.sync.dma_start(out=wo_t[:, :, :], in_=wob_e[:, :, dch * 512:(dch + 1) * 512])
                                wo_ts.append(wo_t)
                    finish(prev)
                    for dch in range(2, 5):
                        wo_t = w3pool.tile([128, NQ, 512], BF16, tag="wo_t")
                        nc.sync.dma_start(out=wo_t[:, :, :], in_=wob_e[:, :, dch * 512:(dch + 1) * 512])
                        wo_ts.append(wo_t)
                    aTe.extend(selects(0, a2aA_out))

                    # ====== Phase 3: AllToAll(B) + o_proj ======
                    nc.gpsimd.collective_compute(
                        "AllToAll",
                        mybir.AluOpType.bypass,
                        replica_groups=[[0, 1, 2, 3, 4, 5, 6, 7]],
                        ins=[a2aB_in[:, :, :, :].opt()],
                        outs=[a2aB_out[:, :, :, :].opt()],
                    )

                    def oproj(tq, aTs, dchs, wts3):
                        for dch in dchs:
                            po = scpool.tile([128, 512], F32, tag="sc")
                            for n in range(NQ):
                                nc.tensor.matmul(
                                    po[:, :],
                                    aTs[n // 8][:, n % 8, :],
                                    wts3[dch][:, n, :],
                                    start=(n == 0),
                                    stop=(n == NQ - 1),
                                )
                            ob = o3pool.tile([128, 512], F32, tag="ob")
                            nc.vector.tensor_copy(ob[:, :], po[:, :])
                            nc.sync.dma_start(
                                out=out_e[tq * 128:(tq + 1) * 128, dch * 512:(dch + 1) * 512],
                                in_=ob[:, :],
                            )

                    # tq=0 (even chunks) depends only on A2A#A -> runs while #B flies
                    oproj(0, aTe, [0, 1, 2, 3, 4], wo_ts)
                    aTo = selects(1, a2aB_out)
                    wo_b = {2: wo_ts[2], 3: wo_ts[3], 4: wo_ts[4]}
                    for dch in (1, 0):
                        wo_t = w3pool.tile([128, NQ, 512], BF16, tag="wo_t")
                        nc.sync.dma_start(out=wo_t[:, :, :], in_=wob_e[:, :, dch * 512:(dch + 1) * 512])
                        wo_b[dch] = wo_t
                    oproj(1, aTo, [4, 3, 2, 1, 0], wo_b)

    return nc


def _rope_tables(pos_b):
    """pos_b: [3, T] int32 -> sin/cos [T, 64] per mrope."""
    fraction = 2.0 * np.arange(0, H // 2, dtype=np.float64) / H
    timescale = ROPE_THETA ** fraction
    sinusoid = pos_b[:, :, None].astype(np.float64) / timescale  # [3, T, 64]
    freq = sinusoid[0].copy()
    h_idx = np.arange(1, MROPE_SECTION[1] * 3, 3)
    w_idx = np.arange(2, MROPE_SECTION[2] * 3, 3)
    freq[:, h_idx] = sinusoid[1][:, h_idx]
    freq[:, w_idx] = sinusoid[2][:, w_idx]
    return np.sin(freq).astype(np.float32), np.cos(freq).astype(np.float32)


def _tables_for(sin, cos, w):
    """C [T,128], S0 [T,64], S1 [T,64] with norm weight w [128] folded in."""
    C = np.concatenate([cos * w[None, :64], cos * w[None, 64:]], axis=1)
    S0 = sin * w[None, 64:]
    S1 = sin * w[None, :64]
    return (np.ascontiguousarray(C), np.ascontiguousarray(S0), np.ascontiguousarray(S1))


def kernel(x, positions, attn_mask, wq, wk, wv, wo, q_norm_w, k_norm_w):
    x = np.asarray(x, dtype=np.float32)
    positions = np.asarray(positions)
    attn_mask = np.asarray(attn_mask)
    wq = np.asarray(wq, dtype=np.float32)
    wk = np.asarray(wk, dtype=np.float32)
    wv = np.asarray(wv, dtype=np.float32)
    wo = np.asarray(wo, dtype=np.float32)
    q_norm_w = np.asarray(q_norm_w, dtype=np.float32)
    k_norm_w = np.asarray(k_norm_w, dtype=np.float32)

    nc = _build_nc()
    nc.finalize()

    identf = np.eye(128, dtype=np.float32)
    identb = np.eye(128).astype(ml_dtypes.bfloat16)
    wob = np.ascontiguousarray(wo.transpose(1, 0, 2)).astype(ml_dtypes.bfloat16)

    in_maps = []
    for c in range(8):
        b, g = c // 4, c % 4
        xt = np.ascontiguousarray(
            x[b].T.reshape(NDT, 128, NTC, 128).transpose(2, 0, 1, 3))
        kvh = slice(g * 2, g * 2 + 2)
        wq0 = np.ascontiguousarray(
            wq[:, g * 8:g * 8 + 4, :].reshape(D, 512).reshape(NDT, 128, 512))
        wq1 = np.ascontiguousarray(
            wq[:, g * 8 + 4:g * 8 + 8, :].reshape(D, 512).reshape(NDT, 128, 512))
        wkv = np.ascontiguousarray(
            np.concatenate([
                wk[:, kvh, :].reshape(D, 256),
                wv[:, kvh, :].reshape(D, 256),
            ], axis=1).reshape(NDT, 128, 512))
        sin, cos = _rope_tables(np.asarray(positions[:, b, :]))
        cqt, s0qt, s1qt = _tables_for(sin, cos, q_norm_w)
        ckt, s0kt, s1kt = _tables_for(sin, cos, k_norm_w)
        mb = np.empty((NTC, 128, 128), np.float32)
        for i in range(NTC):
            blk = attn_mask[b, i * 128:(i + 1) * 128, i * 128:(i + 1) * 128]
            mb[i] = np.where(blk, 0.0, NEG)
        in_maps.append({
            "xt": xt, "wq0": wq0, "wq1": wq1, "wkv": wkv,
            "cq": cqt, "s0q": s0qt, "s1q": s1qt,
            "ck": ckt, "s0k": s0kt, "s1k": s1kt,
            "maska": mb, "identf": identf, "identb": identb, "wob": wob,
            "bsel": np.full((128, 1), 1.0 if b == 0 else 0.0, np.float32),
            "bsm": np.full((128, 1), 0.0 if b == 0 else 1.0, np.float32),
        })

    res = run_bass_kernel_spmd(nc, in_maps, core_ids=list(range(8)))
    global _LAST
    _LAST = res
    full = np.empty((B, T, D), np.float32)
    for c in range(8):
        b, g = c // 4, c % 4
        full[b, g * 256:(g + 1) * 256, :] = res.results[c]["out"]
    return full



# revision 9
# speedup vs baseline: 1.0729x; 1.0729x over previous
"""Distributed Trainium2 kernel for GQA attention block (B=2,T=1024,D=2560,Nq=32,Nkv=8,H=128).

Sharding: 8 cores = 2 batches x 4 head-groups. Core c handles batch c//4 and
q-heads [8g:8g+8), kv-heads [2g:2g+2) where g=c%4. Attention is head-local.
Scores are computed pre-transposed (S.T = kT-block stationary x qT moving) so
AV consumes wide moving operands and needs no per-block PE transposes; softmax
row-sums come from a ones-column matmul, inverted on the scalar engine, and
broadcast to all partitions with a PE outer product.

Two 8-core AllToAlls re-shard heads->tokens: A2A#A carries token chunks 0-3
(fires right after chunks 0-3 are projected + attended, thanks to causality),
A2A#B carries chunks 4-7. Every A2A slot is useful: core (b,g) computes o_proj
columns [1280*b, 1280*b+1280) for BOTH batches' token chunks {g, g+4}, so the
cross-batch shards feed the partner-batch half instead of being discarded.
Output per core: [512, 1280] = (b0 cg, b0 cg+4, b1 cg, b1 cg+4) rows.
"""

import ml_dtypes
import numpy as np

import concourse.bass as bass
import concourse.mybir as mybir
import concourse.tile as tile
from concourse import bacc
from concourse.bass_utils import run_bass_kernel_spmd

F32 = mybir.dt.float32
F32R = mybir.dt.float32r
BF16 = mybir.dt.bfloat16

B, T, D, NQ, NKV, H = 2, 1024, 2560, 32, 8, 128
NDT = D // 128  # 20 contraction tiles
NTC = T // 128  # 8 token chunks
DH = D // 2     # o_proj column half per core
NWO = 5         # o_proj weight sub-tiles of 256 cols
ROPE_THETA = 1000000.0
MROPE_SECTION = (24, 20, 20)
NORM_EPS = 1e-6
SOFT_SCALE = H ** -0.5
NEG = -1e30

EXP_ACT = mybir.ActivationFunctionType.Exp
SQUARE_ACT = mybir.ActivationFunctionType.Square
SQRT_ACT = mybir.ActivationFunctionType.Sqrt
COPY_ACT = mybir.ActivationFunctionType.Copy
MUL_OP = mybir.AluOpType.mult

_LAST = None


def _norm_rope(nc, smpool, tmppool, qrpool, ps, nh, eps_t, ct, s0t, s1t, tagp):
    """RMS-norm over h + rope for nh heads sitting in psum ps[:, :nh*128].

    Returns a BF16 tile [128, nh*128] (token-major) ready for PE transpose.
    The rsqrt scale is fused into the rope products via scalar_tensor_tensor.
    """
    w = nh * 128
    ssq = smpool.tile([128, nh], F32, tag=f"ssq{tagp}")
    for hh in range(nh):
        sq = smpool.tile([128, 128], F32, tag="sq")
        nc.scalar.activation(sq[:, :], ps[:, hh * 128:(hh + 1) * 128], SQUARE_ACT,
                             accum_out=ssq[:, hh:hh + 1])
    srt = smpool.tile([128, nh], F32, tag=f"srt{tagp}")
    nc.scalar.activation(srt[:, :], ssq[:, :], SQRT_ACT, bias=eps_t[:, :], scale=1.0 / H)
    rsq = smpool.tile([128, nh], F32, tag=f"rsq{tagp}")
    nc.vector.reciprocal(rsq[:, :], srt[:, :])
    qm = tmppool.tile([128, w], F32, tag=f"qm{tagp}")
    t1 = tmppool.tile([128, nh * 64], F32, tag=f"t1{tagp}")
    t2 = tmppool.tile([128, nh * 64], F32, tag=f"t2{tagp}")
    qr = qrpool.tile([128, w], BF16, tag=f"qr{tagp}")
    for hh in range(nh):
        rh = rsq[:, hh:hh + 1]
        po = hh * 128
        ho = hh * 64
        # qm = (ps * rsqrt) * (cos*w) ; t1 = (ps_hi * rsqrt) * (sin*w_hi)
        # t2 = (ps_lo * rsqrt) * (sin*w_lo)
        nc.vector.scalar_tensor_tensor(
            qm[:, po:po + 128], ps[:, po:po + 128], rh, ct,
            op0=MUL_OP, op1=MUL_OP)
        nc.vector.scalar_tensor_tensor(
            t1[:, ho:ho + 64], ps[:, po + 64:po + 128], rh, s0t,
            op0=MUL_OP, op1=MUL_OP)
        nc.vector.scalar_tensor_tensor(
            t2[:, ho:ho + 64], ps[:, po:po + 64], rh, s1t,
            op0=MUL_OP, op1=MUL_OP)
    qm4 = qm[:, :].rearrange("p (h x) -> p h x", h=nh)
    qr4 = qr[:, :].rearrange("p (h x) -> p h x", h=nh)
    t14 = t1[:, :].rearrange("p (h x) -> p h x", h=nh)
    t24 = t2[:, :].rearrange("p (h x) -> p h x", h=nh)
    nc.vector.tensor_sub(qr4[:, :, 0:64], qm4[:, :, 0:64], t14)
    nc.vector.tensor_add(qr4[:, :, 64:128], qm4[:, :, 64:128], t24)
    return qr


def _build_nc():
    nc = bacc.Bacc(None, target_bir_lowering=False, num_devices=8)

    xt_e = nc.declare_dram_parameter("xt", [NTC, NDT, 128, 128], BF16, isOutput=False)
    wq0_e = nc.declare_dram_parameter("wq0", [NDT, 128, 512], BF16, isOutput=False)
    wq1_e = nc.declare_dram_parameter("wq1", [NDT, 128, 512], BF16, isOutput=False)
    wkv_e = nc.declare_dram_parameter("wkv", [NDT, 128, 512], BF16, isOutput=False)
    cq_e = nc.declare_dram_parameter("cq", [T, 128], F32, isOutput=False)
    s0q_e = nc.declare_dram_parameter("s0q", [T, 64], F32, isOutput=False)
    s1q_e = nc.declare_dram_parameter("s1q", [T, 64], F32, isOutput=False)
    ck_e = nc.declare_dram_parameter("ck", [T, 128], F32, isOutput=False)
    s0k_e = nc.declare_dram_parameter("s0k", [T, 64], F32, isOutput=False)
    s1k_e = nc.declare_dram_parameter("s1k", [T, 64], F32, isOutput=False)
    maskt_e = nc.declare_dram_parameter("maskt", [NTC, 128, 128], F32, isOutput=False)
    identb_e = nc.declare_dram_parameter("identb", [128, 128], BF16, isOutput=False)
    onesr_e = nc.declare_dram_parameter("onesr", [1, 128], F32R, isOutput=False)
    wob_e = nc.declare_dram_parameter("wob", [NWO, 128, NQ, 256], BF16, isOutput=False)
    out_e = nc.declare_dram_parameter("out", [512, DH], F32, isOutput=True)

    with tile.TileContext(nc) as tc:
        with (
            tc.tile_pool(name="const", bufs=1) as cpool,
            tc.tile_pool(name="dram", bufs=1, space="DRAM") as dpool,
            tc.tile_pool(name="acts", bufs=1) as apool,
        ):
            eps_t = cpool.tile([128, 1], F32, tag="eps")
            nc.gpsimd.memset(eps_t[:, :], NORM_EPS)
            onesc = cpool.tile([128, 1], BF16, tag="onesc")
            nc.gpsimd.memset(onesc[:, :], 1.0)
            onesr = cpool.tile([1, 128], F32R, tag="onesr")
            nc.sync.dma_start(out=onesr[:, :], in_=onesr_e[:, :])
            identb = cpool.tile([128, 128], BF16, tag="identb")
            nc.sync.dma_start(out=identb[:, :], in_=identb_e[:, :])

            # A2A blocks are [hd, head, t]: contiguous on both DMA sides.
            a2aA_in = dpool.tile([8, 128, 8, 128], BF16, tag="a2aA_in")
            a2aA_out = dpool.tile([8, 128, 8, 128], BF16, tag="a2aA_out")
            a2aB_in = dpool.tile([8, 128, 8, 128], BF16, tag="a2aB_in")
            a2aB_out = dpool.tile([8, 128, 8, 128], BF16, tag="a2aB_out")

            qT = apool.tile([128, 8, T], BF16, tag="qT")       # [h, qhead, t]
            kT = apool.tile([128, 2, T], BF16, tag="kT")       # [h, kvhead, t]
            vN = apool.tile([128, NTC, 256], BF16, tag="vN")   # [s_in_chunk, chunk, kv*128+h]
            maskt = apool.tile([128, NTC, 128], F32, tag="maskt")

            pend1 = []

            def make_flush(ptpool):
                def flush1():
                    for qr_, tch_, heads_, dest in pend1:
                        for idx, head in enumerate(heads_):
                            pt = ptpool.tile([128, 128], BF16, tag="pt")
                            nc.tensor.transpose(
                                pt[:, :], qr_[:, idx * 128:(idx + 1) * 128], identb[:, :])
                            nc.vector.tensor_copy(
                                dest[:, head, tch_ * 128:(tch_ + 1) * 128], pt[:, :])
                    pend1.clear()
                return flush1

            # ============ attention for one chunk-group ============
            # Group = 4 query chunks [c0, c0+4). S.T per s-block j, exp,
            # then AV + ones-rowsum matmuls with wide moving operands.
            def attn_group(c0, otg, pools):
                stpool, ovpool, rspool, rbpool, epool, scpool = pools
                jmax = c0 + 4
                pend_epi = []

                def flush_epi():
                    for hq_, ov_, rs_ in pend_epi:
                        rcp = scpool.tile([1, 512], F32R, tag="rcp")
                        with nc.allow_low_precision(reason="f32r is f32 bits"):
                            nc.vector.reciprocal(rcp[:, :], rs_[:, :])
                        rcpb = rbpool.tile([128, 512], F32, tag="rcpb")
                        nc.tensor.matmul(rcpb[:, :], onesr[:, :], rcp[:, :],
                                         start=True, stop=True)
                        rcps = scpool.tile([128, 512], F32, tag="rcps")
                        nc.scalar.activation(rcps[:, :], rcpb[:, :], COPY_ACT)
                        ovv = ov_[:, :].rearrange("p (c t) -> p c t", c=4)
                        rbv = rcps[:, :].rearrange("p (c t) -> p c t", c=4)
                        nc.vector.tensor_mul(otg[:, :, hq_, :], ovv, rbv)
                    pend_epi.clear()

                for hq in range(8):
                    kv = hq // 4
                    ov = ovpool.tile([128, 512], F32, tag="ov")
                    rs = rspool.tile([1, 512], F32, tag="rs")
                    pend_av = []

                    def flush_av():
                        for j_, est_ in pend_av:
                            lo_ = max(j_, c0)
                            co = (lo_ - c0) * 128
                            vt = vN[:, j_, kv * 128:(kv + 1) * 128]
                            if j_ < c0:
                                nc.tensor.matmul(ov[:, :], vt, est_[:, 0:512],
                                                 start=(j_ == 0), stop=False)
                                nc.tensor.matmul(rs[:, :], onesc[:, :], est_[:, 0:512],
                                                 start=(j_ == 0), stop=False)
                            else:
                                st_ = (j_ == 0)
                                # chunk j_ finishes its accumulation now
                                nc.tensor.matmul(ov[:, co:co + 128], vt, est_[:, 0:128],
                                                 start=st_, stop=True)
                                nc.tensor.matmul(rs[:, co:co + 128], onesc[:, :], est_[:, 0:128],
                                                 start=st_, stop=True)
                                if co + 128 < 512:
                                    nc.tensor.matmul(ov[:, co + 128:512], vt,
                                                     est_[:, 128:512 - co],
                                                     start=st_, stop=(j_ == jmax - 1))
                                    nc.tensor.matmul(rs[:, co + 128:512], onesc[:, :],
                                                     est_[:, 128:512 - co],
                                                     start=st_, stop=(j_ == jmax - 1))
                        pend_av.clear()

                    for j in range(jmax):
                        lo_ = max(j, c0)
                        w = (c0 + 4 - lo_) * 128
                        st_t = stpool.tile([128, 512], F32, tag="st")
                        nc.tensor.matmul(
                            st_t[:, 0:w],
                            kT[:, kv, j * 128:(j + 1) * 128],
                            qT[:, hq, lo_ * 128:(c0 + 4) * 128],
                            start=True, stop=True,
                        )
                        if j >= c0:
                            nc.vector.tensor_add(st_t[:, 0:128], st_t[:, 0:128], maskt[:, j, :])
                        est = epool.tile([128, 512], BF16, tag="est")
                        nc.scalar.activation(est[:, 0:w], st_t[:, 0:w], EXP_ACT, scale=SOFT_SCALE)
                        flush_av()
                        if hq > 0 and j == 1:
                            flush_epi()
                        pend_av.append((j, est))
                    flush_av()
                    pend_epi.append((hq, ov, rs))
                flush_epi()

            # ================= Phase 1 + attention =================
            with (
                tc.tile_pool(name="p1tab", bufs=1) as tabpool,
                tc.tile_pool(name="p1x", bufs=2) as xpool,
                tc.tile_pool(name="p1w", bufs=60) as wpool,
                tc.tile_pool(name="p1qr", bufs=2) as qrpool,
                tc.tile_pool(name="p1tmp", bufs=1) as tmppool,
                tc.tile_pool(name="p1sm", bufs=2) as smpool,
            ):
                wkvts, wq0ts, wq1ts = [], [], []
                for d in range(NDT):
                    wt = wpool.tile([128, 512], BF16, tag="wt")
                    nc.sync.dma_start(out=wt[:, :], in_=wkv_e[d, :, :])
                    wkvts.append(wt)
                for d in range(NDT):
                    wt = wpool.tile([128, 512], BF16, tag="wt")
                    nc.sync.dma_start(out=wt[:, :], in_=wq0_e[d, :, :])
                    wq0ts.append(wt)
                for d in range(NDT):
                    wt = wpool.tile([128, 512], BF16, tag="wt")
                    nc.sync.dma_start(out=wt[:, :], in_=wq1_e[d, :, :])
                    wq1ts.append(wt)

                def ld(name, shp, src, rearr):
                    t = tabpool.tile(shp, F32, tag=name)
                    nc.sync.dma_start(out=t[:, :, :], in_=src[:, :].rearrange(rearr, p=128))
                    return t

                cq = ld("cq", [128, NTC, 128], cq_e, "(c p) m -> p c m")
                s0q = ld("s0q", [128, NTC, 64], s0q_e, "(c p) m -> p c m")
                s1q = ld("s1q", [128, NTC, 64], s1q_e, "(c p) m -> p c m")
                ck = ld("ck", [128, NTC, 128], ck_e, "(c p) m -> p c m")
                s0k = ld("s0k", [128, NTC, 64], s0k_e, "(c p) m -> p c m")
                s1k = ld("s1k", [128, NTC, 64], s1k_e, "(c p) m -> p c m")
                nc.sync.dma_start(out=maskt[:, :, :],
                                  in_=maskt_e[:, :, :].rearrange("i p m -> p i m"))

                def proj_chunks(chunks, pspool, flush1):
                    for tch in chunks:
                        xc = xpool.tile([128, NDT, 128], BF16, tag="xc")
                        nc.sync.dma_start(
                            out=xc[:, :, :],
                            in_=xt_e[tch, :, :, :].rearrange("d p t -> p d t"))
                        pskv = pspool.tile([128, 512], F32, tag="pskv")
                        for d in range(NDT):
                            nc.tensor.matmul(pskv[:, :], xc[:, d, :], wkvts[d][:, :],
                                             start=(d == 0), stop=(d == NDT - 1))
                        ps0 = pspool.tile([128, 512], F32, tag="ps0")
                        for d in range(NDT):
                            nc.tensor.matmul(ps0[:, :], xc[:, d, :], wq0ts[d][:, :],
                                             start=(d == 0), stop=(d == NDT - 1))
                        ps1 = pspool.tile([128, 512], F32, tag="ps1")
                        for d in range(NDT):
                            nc.tensor.matmul(ps1[:, :], xc[:, d, :], wq1ts[d][:, :],
                                             start=(d == 0), stop=(d == NDT - 1))
                        flush1()
                        kr = _norm_rope(nc, smpool, tmppool, qrpool, pskv[:, 0:256], 2,
                                        eps_t, ck[:, tch, :], s0k[:, tch, :], s1k[:, tch, :], "k")
                        pend1.append((kr, tch, [0, 1], kT))
                        nc.vector.tensor_copy(vN[:, tch, :], pskv[:, 256:512])
                        qr0 = _norm_rope(nc, smpool, tmppool, qrpool, ps0[:, :], 4,
                                         eps_t, cq[:, tch, :], s0q[:, tch, :], s1q[:, tch, :], "q0")
                        pend1.append((qr0, tch, [0, 1, 2, 3], qT))
                        qr1 = _norm_rope(nc, smpool, tmppool, qrpool, ps1[:, :], 4,
                                         eps_t, cq[:, tch, :], s0q[:, tch, :], s1q[:, tch, :], "q1")
                        pend1.append((qr1, tch, [4, 5, 6, 7], qT))

                with (
                    tc.tile_pool(name="ppA", bufs=2, space="PSUM") as pspoolA,
                    tc.tile_pool(name="ptA", bufs=2, space="PSUM") as ptpoolA,
                ):
                    fl = make_flush(ptpoolA)
                    proj_chunks([0, 1, 2, 3], pspoolA, fl)
                    fl()

                with (
                    tc.tile_pool(name="stA", bufs=2, space="PSUM") as stp,
                    tc.tile_pool(name="ovA", bufs=2, space="PSUM") as ovp,
                    tc.tile_pool(name="rsA", bufs=2, space="PSUM") as rsp,
                    tc.tile_pool(name="rbA", bufs=1, space="PSUM") as rbp,
                    tc.tile_pool(name="eA", bufs=2) as ep,
                    tc.tile_pool(name="scA", bufs=2) as scp,
                    tc.tile_pool(name="otA", bufs=1) as otp,
                ):
                    otgA = otp.tile([128, 4, 8, 128], BF16, tag="otgA")
                    attn_group(0, otgA, (stp, ovp, rsp, rbp, ep, scp))
                    for s in range(8):
                        nc.sync.dma_start(out=a2aA_in[s, :, :, :], in_=otgA[:, s % 4, :, :])
                    nc.gpsimd.collective_compute(
                        "AllToAll", mybir.AluOpType.bypass,
                        replica_groups=[[0, 1, 2, 3, 4, 5, 6, 7]],
                        ins=[a2aA_in[:, :, :, :].opt()],
                        outs=[a2aA_out[:, :, :, :].opt()],
                    )

                with (
                    tc.tile_pool(name="ppB", bufs=2, space="PSUM") as pspoolB,
                    tc.tile_pool(name="ptB", bufs=2, space="PSUM") as ptpoolB,
                ):
                    fl = make_flush(ptpoolB)
                    proj_chunks([4, 5, 6, 7], pspoolB, fl)
                    fl()

            # proj sbuf pools closed; o_proj weights load into freed space
            with (
                tc.tile_pool(name="wo3", bufs=NWO) as wopool,
                tc.tile_pool(name="p3a", bufs=1) as a3pool,
                tc.tile_pool(name="p3o", bufs=3) as o3pool,
            ):
                wo_ts = []
                for c in range(NWO):
                    wo_t = wopool.tile([128, NQ, 256], BF16, tag="wo_t")
                    nc.gpsimd.dma_start(out=wo_t[:, :, :], in_=wob_e[c, :, :, :])
                    wo_ts.append(wo_t)

                with (
                    tc.tile_pool(name="stB", bufs=2, space="PSUM") as stp,
                    tc.tile_pool(name="ovB", bufs=2, space="PSUM") as ovp,
                    tc.tile_pool(name="rsB", bufs=2, space="PSUM") as rsp,
                    tc.tile_pool(name="rbB", bufs=1, space="PSUM") as rbp,
                    tc.tile_pool(name="eB", bufs=2) as ep,
                    tc.tile_pool(name="scB", bufs=2) as scp,
                    tc.tile_pool(name="otB", bufs=1) as otp,
                ):
                    otgB = otp.tile([128, 4, 8, 128], BF16, tag="otgB")
                    attn_group(4, otgB, (stp, ovp, rsp, rbp, ep, scp))
                    for s in range(8):
                        nc.sync.dma_start(out=a2aB_in[s, :, :, :], in_=otgB[:, s % 4, :, :])
                    nc.gpsimd.collective_compute(
                        "AllToAll", mybir.AluOpType.bypass,
                        replica_groups=[[0, 1, 2, 3, 4, 5, 6, 7]],
                        ins=[a2aB_in[:, :, :, :].opt()],
                        outs=[a2aB_out[:, :, :, :].opt()],
                    )

                # ============ Phase 3: o_proj (both batches, my D-half) ====
                with tc.tile_pool(name="poP", bufs=2, space="PSUM") as popool:
                    def oproj(par, a2a_out_t):
                        # aT[bb][g'] = heads 8g'..8g'+8 of batch bb, my chunk
                        aTs = []
                        for s in range(8):
                            aT = a3pool.tile([128, 8, 128], BF16, tag=f"aT{par}{s}")
                            nc.gpsimd.dma_start(out=aT[:, :, :], in_=a2a_out_t[s, :, :, :])
                            aTs.append(aT)
                        for bb in range(2):
                            for c in range(NWO):
                                po = popool.tile([128, 256], F32, tag="po")
                                for n in range(NQ):
                                    nc.tensor.matmul(
                                        po[:, :],
                                        aTs[4 * bb + n // 8][:, n % 8, :],
                                        wo_ts[c][:, n, :],
                                        start=(n == 0), stop=(n == NQ - 1))
                                ob = o3pool.tile([128, 256], F32, tag="ob")
                                nc.vector.tensor_copy(ob[:, :], po[:, :])
                                ro = (2 * bb + par) * 128
                                nc.sync.dma_start(
                                    out=out_e[ro:ro + 128, c * 256:(c + 1) * 256],
                                    in_=ob[:, :])

                    oproj(0, a2aA_out)
                    oproj(1, a2aB_out)

    return nc


def _rope_tables(pos_b):
    """pos_b: [3, T] int32 -> sin/cos [T, 64] per mrope."""
    fraction = 2.0 * np.arange(0, H // 2, dtype=np.float64) / H
    timescale = ROPE_THETA ** fraction
    sinusoid = pos_b[:, :, None].astype(np.float64) / timescale  # [3, T, 64]
    freq = sinusoid[0].copy()
    h_idx = np.arange(1, MROPE_SECTION[1] * 3, 3)
    w_idx = np.arange(2, MROPE_SECTION[2] * 3, 3)
    freq[:, h_idx] = sinusoid[1][:, h_idx]
    freq[:, w_idx] = sinusoid[2][:, w_idx]
    return np.sin(freq).astype(np.float32), np.cos(freq).astype(np.float32)


def _tables_for(sin, cos, w):
    """C [T,128], S0 [T,64], S1 [T,64] with norm weight w [128] folded in."""
    C = np.concatenate([cos * w[None, :64], cos * w[None, 64:]], axis=1)
    S0 = sin * w[None, 64:]
    S1 = sin * w[None, :64]
    return (np.ascontiguousarray(C), np.ascontiguousarray(S0), np.ascontiguousarray(S1))


def kernel(x, positions, attn_mask, wq, wk, wv, wo, q_norm_w, k_norm_w):
    x = np.asarray(x, dtype=np.float32)
    positions = np.asarray(positions)
    attn_mask = np.asarray(attn_mask)
    wq = np.asarray(wq, dtype=np.float32)
    wk = np.asarray(wk, dtype=np.float32)
    wv = np.asarray(wv, dtype=np.float32)
    wo = np.asarray(wo, dtype=np.float32)
    q_norm_w = np.asarray(q_norm_w, dtype=np.float32)
    k_norm_w = np.asarray(k_norm_w, dtype=np.float32)

    nc = _build_nc()
    nc.finalize()

    identb = np.eye(128).astype(ml_dtypes.bfloat16)
    wot = wo.transpose(1, 0, 2)  # [hd=128, n=32, d]

    in_maps = []
    for c in range(8):
        b, g = c // 4, c % 4
        xt = np.ascontiguousarray(
            x[b].T.reshape(NDT, 128, NTC, 128).transpose(2, 0, 1, 3)
        ).astype(ml_dtypes.bfloat16)
        kvh = slice(g * 2, g * 2 + 2)
        wq0 = np.ascontiguousarray(
            wq[:, g * 8:g * 8 + 4, :].reshape(D, 512).reshape(NDT, 128, 512)
        ).astype(ml_dtypes.bfloat16)
        wq1 = np.ascontiguousarray(
            wq[:, g * 8 + 4:g * 8 + 8, :].reshape(D, 512).reshape(NDT, 128, 512)
        ).astype(ml_dtypes.bfloat16)
        wkv = np.ascontiguousarray(
            np.concatenate([
                wk[:, kvh, :].reshape(D, 256),
                wv[:, kvh, :].reshape(D, 256),
            ], axis=1).reshape(NDT, 128, 512)
        ).astype(ml_dtypes.bfloat16)
        sin, cos = _rope_tables(np.asarray(positions[:, b, :]))
        cqt, s0qt, s1qt = _tables_for(sin, cos, q_norm_w)
        ckt, s0kt, s1kt = _tables_for(sin, cos, k_norm_w)
        mb = np.empty((NTC, 128, 128), np.float32)
        for i in range(NTC):
            blk = attn_mask[b, i * 128:(i + 1) * 128, i * 128:(i + 1) * 128]
            mb[i] = np.where(blk.T, 0.0, NEG)  # [s, t] orientation
        wob = np.ascontiguousarray(
            wot[:, :, b * DH:(b + 1) * DH].reshape(128, NQ, NWO, 256)
            .transpose(2, 0, 1, 3)).astype(ml_dtypes.bfloat16)
        in_maps.append({
            "xt": xt, "wq0": wq0, "wq1": wq1, "wkv": wkv,
            "cq": cqt, "s0q": s0qt, "s1q": s1qt,
            "ck": ckt, "s0k": s0kt, "s1k": s1kt,
            "maskt": mb, "identb": identb, "wob": wob,
            "onesr": np.ones((1, 128), np.float32),
        })

    res = run_bass_kernel_spmd(nc, in_maps, core_ids=list(range(8)))
    global _LAST
    _LAST = res
    full = np.empty((B, T, D), np.float32)
    for c in range(8):
        b, g = c // 4, c % 4
        r = res.results[c]["out"]  # [512, 1280]
        cols = slice(b * DH, (b + 1) * DH)
        full[0, g * 128:(g + 1) * 128, cols] = r[0:128]
        full[0, (g + 4) * 128:(g + 5) * 128, cols] = r[128:256]
        full[1, g * 128:(g + 1) * 128, cols] = r[256:384]
        full[1, (g + 4) * 128:(g + 5) * 128, cols] = r[384:512]
    return full


# revision 15
# speedup vs baseline: 1.3248x; 1.2348x over previous
"""Distributed Trainium2 kernel for GQA attention block (B=2,T=1024,D=2560,Nq=32,Nkv=8,H=128).

Sharding: 8 cores = 2 batches x 4 head-groups. Core c handles batch c//4 and
q-heads [8g:8g+8), kv-heads [2g:2g+2) where g=c%4. Attention is head-local.
Scores are computed pre-transposed (S.T = kT-block stationary x qT moving) so
AV consumes wide moving operands and needs no per-block PE transposes; softmax
row-sums come from a ones-column matmul, inverted on the scalar engine, and
broadcast to all partitions with a PE outer product.

Two 8-core AllToAlls re-shard heads->tokens: A2A#A carries token chunks 0-3
(fires right after chunks 0-3 are projected + attended, thanks to causality),
A2A#B carries chunks 4-7. Every A2A slot is useful: core (b,g) computes o_proj
columns [1280*b, 1280*b+1280) for BOTH batches' token chunks {g, g+4}, so the
cross-batch shards feed the partner-batch half instead of being discarded.
Output per core: [512, 1280] = (b0 cg, b0 cg+4, b1 cg, b1 cg+4) rows.
"""

import ml_dtypes
import numpy as np

import concourse.bass as bass
import concourse.mybir as mybir
import concourse.tile as tile
from concourse import bacc
from concourse.bass_utils import run_bass_kernel_spmd

F32 = mybir.dt.float32
F32R = mybir.dt.float32r
BF16 = mybir.dt.bfloat16

B, T, D, NQ, NKV, H = 2, 1024, 2560, 32, 8, 128
NDT = D // 128  # 20 contraction tiles
NTC = T // 128  # 8 token chunks
DH = D // 2     # o_proj column half per core
NWO = 5         # o_proj weight sub-tiles of 256 cols
ROPE_THETA = 1000000.0
MROPE_SECTION = (24, 20, 20)
NORM_EPS = 1e-6
SOFT_SCALE = H ** -0.5
NEG = -1e30

EXP_ACT = mybir.ActivationFunctionType.Exp
SQUARE_ACT = mybir.ActivationFunctionType.Square
SQRT_ACT = mybir.ActivationFunctionType.Sqrt
COPY_ACT = mybir.ActivationFunctionType.Copy
MUL_OP = mybir.AluOpType.mult

_LAST = None


def _norm_rope(nc, smpool, tmppool, qrpool, ps, nh, eps_t, ct, s0t, s1t, tagp):
    """RMS-norm over h + rope for nh heads sitting in psum ps[:, :nh*128].

    Returns a BF16 tile [128, nh*128] (token-major) ready for PE transpose.
    The rsqrt scale is fused into the rope products via scalar_tensor_tensor.
    """
    w = nh * 128
    ssq = smpool.tile([128, nh], F32, tag=f"ssq{tagp}")
    for hh in range(nh):
        sq = smpool.tile([128, 128], F32, tag="sq")
        nc.scalar.activation(sq[:, :], ps[:, hh * 128:(hh + 1) * 128], SQUARE_ACT,
                             accum_out=ssq[:, hh:hh + 1])
    srt = smpool.tile([128, nh], F32, tag=f"srt{tagp}")
    nc.scalar.activation(srt[:, :], ssq[:, :], SQRT_ACT, bias=eps_t[:, :], scale=1.0 / H)
    rsq = smpool.tile([128, nh], F32, tag=f"rsq{tagp}")
    nc.vector.reciprocal(rsq[:, :], srt[:, :])
    qm = tmppool.tile([128, w], F32, tag=f"qm{tagp}")
    t1 = tmppool.tile([128, nh * 64], F32, tag=f"t1{tagp}")
    t2 = tmppool.tile([128, nh * 64], F32, tag=f"t2{tagp}")
    qr = qrpool.tile([128, w], BF16, tag=f"qr{tagp}")
    for hh in range(nh):
        rh = rsq[:, hh:hh + 1]
        po = hh * 128
        ho = hh * 64
        # qm = (ps * rsqrt) * (cos*w) ; t1 = (ps_hi * rsqrt) * (sin*w_hi)
        # t2 = (ps_lo * rsqrt) * (sin*w_lo)
        nc.vector.scalar_tensor_tensor(
            qm[:, po:po + 128], ps[:, po:po + 128], rh, ct,
            op0=MUL_OP, op1=MUL_OP)
        nc.vector.scalar_tensor_tensor(
            t1[:, ho:ho + 64], ps[:, po + 64:po + 128], rh, s0t,
            op0=MUL_OP, op1=MUL_OP)
        nc.vector.scalar_tensor_tensor(
            t2[:, ho:ho + 64], ps[:, po:po + 64], rh, s1t,
            op0=MUL_OP, op1=MUL_OP)
    qm4 = qm[:, :].rearrange("p (h x) -> p h x", h=nh)
    qr4 = qr[:, :].rearrange("p (h x) -> p h x", h=nh)
    t14 = t1[:, :].rearrange("p (h x) -> p h x", h=nh)
    t24 = t2[:, :].rearrange("p (h x) -> p h x", h=nh)
    nc.vector.tensor_sub(qr4[:, :, 0:64], qm4[:, :, 0:64], t14)
    nc.vector.tensor_add(qr4[:, :, 64:128], qm4[:, :, 64:128], t24)
    return qr


def _build_nc():
    nc = bacc.Bacc(None, target_bir_lowering=False, num_devices=8)

    xt_e = nc.declare_dram_parameter("xt", [NTC, NDT, 128, 128], BF16, isOutput=False)
    wq0_e = nc.declare_dram_parameter("wq0", [NDT, 128, 512], BF16, isOutput=False)
    wq1_e = nc.declare_dram_parameter("wq1", [NDT, 128, 512], BF16, isOutput=False)
    wkv_e = nc.declare_dram_parameter("wkv", [NDT, 128, 512], BF16, isOutput=False)
    cq_e = nc.declare_dram_parameter("cq", [T, 128], F32, isOutput=False)
    s0q_e = nc.declare_dram_parameter("s0q", [T, 64], F32, isOutput=False)
    s1q_e = nc.declare_dram_parameter("s1q", [T, 64], F32, isOutput=False)
    ck_e = nc.declare_dram_parameter("ck", [T, 128], F32, isOutput=False)
    s0k_e = nc.declare_dram_parameter("s0k", [T, 64], F32, isOutput=False)
    s1k_e = nc.declare_dram_parameter("s1k", [T, 64], F32, isOutput=False)
    maskt_e = nc.declare_dram_parameter("maskt", [NTC, 128, 128], F32, isOutput=False)
    identb_e = nc.declare_dram_parameter("identb", [128, 128], BF16, isOutput=False)
    onesr_e = nc.declare_dram_parameter("onesr", [1, 128], F32R, isOutput=False)
    wob_e = nc.declare_dram_parameter("wob", [NWO, 128, NQ, 256], BF16, isOutput=False)
    out_e = nc.declare_dram_parameter("out", [512, DH], F32, isOutput=True)

    with tile.TileContext(nc) as tc:
        with (
            tc.tile_pool(name="const", bufs=1) as cpool,
            tc.tile_pool(name="dram", bufs=1, space="DRAM") as dpool,
            tc.tile_pool(name="acts", bufs=1) as apool,
        ):
            eps_t = cpool.tile([128, 1], F32, tag="eps")
            nc.gpsimd.memset(eps_t[:, :], NORM_EPS)
            onesc = cpool.tile([128, 1], BF16, tag="onesc")
            nc.gpsimd.memset(onesc[:, :], 1.0)
            onesr = cpool.tile([1, 128], F32R, tag="onesr")
            nc.sync.dma_start(out=onesr[:, :], in_=onesr_e[:, :])
            identb = cpool.tile([128, 128], BF16, tag="identb")
            nc.sync.dma_start(out=identb[:, :], in_=identb_e[:, :])

            # A2A blocks are [hd, head, t]: contiguous on both DMA sides.
            a2aA_in = dpool.tile([8, 128, 8, 128], BF16, tag="a2aA_in")
            a2aA_out = dpool.tile([8, 128, 8, 128], BF16, tag="a2aA_out")
            a2aB_in = dpool.tile([8, 128, 8, 128], BF16, tag="a2aB_in")
            a2aB_out = dpool.tile([8, 128, 8, 128], BF16, tag="a2aB_out")

            qT = apool.tile([128, 8, T], BF16, tag="qT")       # [h, qhead, t]
            kT = apool.tile([128, 2, T], BF16, tag="kT")       # [h, kvhead, t]
            vN = apool.tile([128, NTC, 256], BF16, tag="vN")   # [s_in_chunk, chunk, kv*128+h]
            maskt = apool.tile([128, NTC, 128], F32, tag="maskt")

            pend1 = []

            def make_flush(ptpool):
                def flush1():
                    for qr_, tch_, heads_, dest in pend1:
                        for idx, head in enumerate(heads_):
                            pt = ptpool.tile([128, 128], BF16, tag="pt")
                            nc.tensor.transpose(
                                pt[:, :], qr_[:, idx * 128:(idx + 1) * 128], identb[:, :])
                            nc.vector.tensor_copy(
                                dest[:, head, tch_ * 128:(tch_ + 1) * 128], pt[:, :])
                    pend1.clear()
                return flush1

            # ============ attention for one chunk-group ============
            # Group = 4 query chunks [c0, c0+4). S.T per s-block j, exp,
            # then AV + ones-rowsum matmuls with wide moving operands.
            def attn_group(c0, otg, pools):
                stpool, ovpool, rspool, rbpool, epool, scpool = pools
                jmax = c0 + 4
                pend_epi = []
                pend_av = []

                def flush_epi():
                    for hq_, ov_, rs_ in pend_epi:
                        # rowsum -> sbuf (scalar), broadcast via PE outer
                        # product, reciprocal across all 128 partitions (DVE).
                        rss = scpool.tile([1, 512], F32R, tag="rss")
                        nc.scalar.activation(rss[:, :], rs_[:, :], COPY_ACT)
                        sumb = rbpool.tile([128, 512], F32, tag="sumb")
                        nc.tensor.matmul(sumb[:, :], onesr[:, :], rss[:, :],
                                         start=True, stop=True)
                        rcps = scpool.tile([128, 512], F32, tag="rcps")
                        nc.vector.reciprocal_approx_fast(rcps[:, :], sumb[:, :])
                        ovv = ov_[:, :].rearrange("p (c t) -> p c t", c=4)
                        rbv = rcps[:, :].rearrange("p (c t) -> p c t", c=4)
                        nc.vector.tensor_mul(otg[:, :, hq_, :], ovv, rbv)
                    pend_epi.clear()

                def flush_av():
                    j_, est_, ov, rs, kv = pend_av.pop(0)
                    lo_ = max(j_, c0)
                    co = (lo_ - c0) * 128
                    vt = vN[:, j_, kv * 128:(kv + 1) * 128]
                    if j_ < c0:
                        nc.tensor.matmul(ov[:, :], vt, est_[:, 0:512],
                                         start=(j_ == 0), stop=False)
                        nc.tensor.matmul(rs[:, :], onesc[:, :], est_[:, 0:512],
                                         start=(j_ == 0), stop=False)
                    else:
                        st_ = (j_ == 0)
                        # chunk j_ finishes its accumulation now
                        nc.tensor.matmul(ov[:, co:co + 128], vt, est_[:, 0:128],
                                         start=st_, stop=True)
                        nc.tensor.matmul(rs[:, co:co + 128], onesc[:, :], est_[:, 0:128],
                                         start=st_, stop=True)
                        if co + 128 < 512:
                            nc.tensor.matmul(ov[:, co + 128:512], vt,
                                             est_[:, 128:512 - co],
                                             start=st_, stop=(j_ == jmax - 1))
                            nc.tensor.matmul(rs[:, co + 128:512], onesc[:, :],
                                             est_[:, 128:512 - co],
                                             start=st_, stop=(j_ == jmax - 1))

                for hq in range(8):
                    kv = hq // 4
                    ov = ovpool.tile([128, 512], F32, tag="ov")
                    rs = rspool.tile([1, 512], F32, tag="rs")
                    for j in range(jmax):
                        lo_ = max(j, c0)
                        w = (c0 + 4 - lo_) * 128
                        st_t = stpool.tile([128, 512], F32, tag="st")
                        nc.tensor.matmul(
                            st_t[:, 0:w],
                            kT[:, kv, j * 128:(j + 1) * 128],
                            qT[:, hq, lo_ * 128:(c0 + 4) * 128],
                            start=True, stop=True,
                        )
                        if j >= c0:
                            nc.vector.tensor_add(st_t[:, 0:128], st_t[:, 0:128], maskt[:, j, :])
                        est = epool.tile([128, 512], BF16, tag="est")
                        nc.scalar.activation(est[:, 0:w], st_t[:, 0:w], EXP_ACT, scale=SOFT_SCALE)
                        while len(pend_av) >= 2:
                            flush_av()
                        if hq > 0 and j == 1:
                            flush_epi()
                        pend_av.append((j, est, ov, rs, kv))
                    pend_epi.append((hq, ov, rs))
                while pend_av:
                    flush_av()
                flush_epi()

            # ================= Phase 1 + attention =================
            with (
                tc.tile_pool(name="p1tab", bufs=1) as tabpool,
                tc.tile_pool(name="p1x", bufs=3) as xpool,
                tc.tile_pool(name="p1w", bufs=60) as wpool,
                tc.tile_pool(name="p1qr", bufs=2) as qrpool,
                tc.tile_pool(name="p1tmp", bufs=1) as tmppool,
                tc.tile_pool(name="p1sm", bufs=2) as smpool,
            ):
                # all x chunk tiles are DMA'd upfront on the gpsimd queue so
                # weight loads on the sync queue never starve them; the pool
                # recycling (bufs=3) paces the later chunks automatically
                xcs = {}
                for tch in range(NTC):
                    xc = xpool.tile([128, NDT, 128], BF16, tag="xc")
                    nc.gpsimd.dma_start(
                        out=xc[:, :, :],
                        in_=xt_e[tch, :, :, :].rearrange("d p t -> p d t"))
                    xcs[tch] = xc
                def ld(name, shp, src, rearr):
                    t = tabpool.tile(shp, F32, tag=name)
                    nc.sync.dma_start(out=t[:, :, :], in_=src[:, :].rearrange(rearr, p=128))
                    return t

                cq = ld("cq", [128, NTC, 128], cq_e, "(c p) m -> p c m")
                s0q = ld("s0q", [128, NTC, 64], s0q_e, "(c p) m -> p c m")
                s1q = ld("s1q", [128, NTC, 64], s1q_e, "(c p) m -> p c m")
                ck = ld("ck", [128, NTC, 128], ck_e, "(c p) m -> p c m")
                s0k = ld("s0k", [128, NTC, 64], s0k_e, "(c p) m -> p c m")
                s1k = ld("s1k", [128, NTC, 64], s1k_e, "(c p) m -> p c m")
                nc.sync.dma_start(out=maskt[:, :, :],
                                  in_=maskt_e[:, :, :].rearrange("i p m -> p i m"))

                wkvts, wq0ts, wq1ts = [], [], []
                for d in range(NDT):
                    wt = wpool.tile([128, 512], BF16, tag="wt")
                    nc.sync.dma_start(out=wt[:, :], in_=wkv_e[d, :, :])
                    wkvts.append(wt)
                for d in range(NDT):
                    wt = wpool.tile([128, 512], BF16, tag="wt")
                    nc.sync.dma_start(out=wt[:, :], in_=wq0_e[d, :, :])
                    wq0ts.append(wt)
                for d in range(NDT):
                    wt = wpool.tile([128, 512], BF16, tag="wt")
                    nc.sync.dma_start(out=wt[:, :], in_=wq1_e[d, :, :])
                    wq1ts.append(wt)

                def proj_chunks(chunks, pspool, flush1):
                    for tch in chunks:
                        xc = xcs.pop(tch)
                        pskv = pspool.tile([128, 512], F32, tag="pskv")
                        for d in range(NDT):
                            nc.tensor.matmul(pskv[:, :], xc[:, d, :], wkvts[d][:, :],
                                             start=(d == 0), stop=(d == NDT - 1))
                        ps0 = pspool.tile([128, 512], F32, tag="ps0")
                        for d in range(NDT):
                            nc.tensor.matmul(ps0[:, :], xc[:, d, :], wq0ts[d][:, :],
                                             start=(d == 0), stop=(d == NDT - 1))
                        ps1 = pspool.tile([128, 512], F32, tag="ps1")
                        for d in range(NDT):
                            nc.tensor.matmul(ps1[:, :], xc[:, d, :], wq1ts[d][:, :],
                                             start=(d == 0), stop=(d == NDT - 1))
                        flush1()
                        kr = _norm_rope(nc, smpool, tmppool, qrpool, pskv[:, 0:256], 2,
                                        eps_t, ck[:, tch, :], s0k[:, tch, :], s1k[:, tch, :], "k")
                        pend1.append((kr, tch, [0, 1], kT))
                        nc.vector.tensor_copy(vN[:, tch, :], pskv[:, 256:512])
                        qr0 = _norm_rope(nc, smpool, tmppool, qrpool, ps0[:, :], 4,
                                         eps_t, cq[:, tch, :], s0q[:, tch, :], s1q[:, tch, :], "q0")
                        pend1.append((qr0, tch, [0, 1, 2, 3], qT))
                        qr1 = _norm_rope(nc, smpool, tmppool, qrpool, ps1[:, :], 4,
                                         eps_t, cq[:, tch, :], s0q[:, tch, :], s1q[:, tch, :], "q1")
                        pend1.append((qr1, tch, [4, 5, 6, 7], qT))

                with (
                    tc.tile_pool(name="ppA", bufs=2, space="PSUM") as pspoolA,
                    tc.tile_pool(name="ptA", bufs=2, space="PSUM") as ptpoolA,
                ):
                    fl = make_flush(ptpoolA)
                    proj_chunks([0, 1, 2, 3], pspoolA, fl)
                    fl()

                with (
                    tc.tile_pool(name="stA", bufs=3, space="PSUM") as stp,
                    tc.tile_pool(name="ovA", bufs=2, space="PSUM") as ovp,
                    tc.tile_pool(name="rsA", bufs=2, space="PSUM") as rsp,
                    tc.tile_pool(name="rbA", bufs=1, space="PSUM") as rbp,
                    tc.tile_pool(name="eA", bufs=3) as ep,
                    tc.tile_pool(name="scA", bufs=2) as scp,
                    tc.tile_pool(name="otA", bufs=1) as otp,
                ):
                    otgA = otp.tile([128, 4, 8, 128], BF16, tag="otgA")
                    attn_group(0, otgA, (stp, ovp, rsp, rbp, ep, scp))
                    for s in range(8):
                        nc.sync.dma_start(out=a2aA_in[s, :, :, :], in_=otgA[:, s % 4, :, :])
                    nc.gpsimd.collective_compute(
                        "AllToAll", mybir.AluOpType.bypass,
                        replica_groups=[[0, 1, 2, 3, 4, 5, 6, 7]],
                        ins=[a2aA_in[:, :, :, :].opt()],
                        outs=[a2aA_out[:, :, :, :].opt()],
                    )

                with (
                    tc.tile_pool(name="ppB", bufs=2, space="PSUM") as pspoolB,
                    tc.tile_pool(name="ptB", bufs=2, space="PSUM") as ptpoolB,
                ):
                    fl = make_flush(ptpoolB)
                    proj_chunks([4, 5, 6, 7], pspoolB, fl)
                    fl()

            # proj sbuf pools closed; o_proj weights load into freed space
            with (
                tc.tile_pool(name="wo3", bufs=NWO) as wopool,
                tc.tile_pool(name="p3a", bufs=1) as a3pool,
                tc.tile_pool(name="p3o", bufs=3) as o3pool,
            ):
                wo_ts = []
                for c in range(NWO):
                    wo_t = wopool.tile([128, NQ, 256], BF16, tag="wo_t")
                    nc.gpsimd.dma_start(out=wo_t[:, :, :], in_=wob_e[c, :, :, :])
                    wo_ts.append(wo_t)

                with (
                    tc.tile_pool(name="stB", bufs=3, space="PSUM") as stp,
                    tc.tile_pool(name="ovB", bufs=2, space="PSUM") as ovp,
                    tc.tile_pool(name="rsB", bufs=2, space="PSUM") as rsp,
                    tc.tile_pool(name="rbB", bufs=1, space="PSUM") as rbp,
                    tc.tile_pool(name="eB", bufs=3) as ep,
                    tc.tile_pool(name="scB", bufs=2) as scp,
                    tc.tile_pool(name="otB", bufs=1) as otp,
                ):
                    otgB = otp.tile([128, 4, 8, 128], BF16, tag="otgB")
                    attn_group(4, otgB, (stp, ovp, rsp, rbp, ep, scp))
                    for s in range(8):
                        nc.sync.dma_start(out=a2aB_in[s, :, :, :], in_=otgB[:, s % 4, :, :])
                    nc.gpsimd.collective_compute(
                        "AllToAll", mybir.AluOpType.bypass,
                        replica_groups=[[0, 1, 2, 3, 4, 5, 6, 7]],
                        ins=[a2aB_in[:, :, :, :].opt()],
                        outs=[a2aB_out[:, :, :, :].opt()],
                    )

                # ============ Phase 3: o_proj (both batches, my D-half) ====
                with tc.tile_pool(name="poP", bufs=2, space="PSUM") as popool:
                    def oproj(par, a2a_out_t):
                        # aT[bb][g'] = heads 8g'..8g'+8 of batch bb, my chunk
                        aTs = []
                        for s in range(8):
                            aT = a3pool.tile([128, 8, 128], BF16, tag=f"aT{par}{s}")
                            nc.gpsimd.dma_start(out=aT[:, :, :], in_=a2a_out_t[s, :, :, :])
                            aTs.append(aT)
                        for bb in range(2):
                            for c in range(NWO):
                                po = popool.tile([128, 256], F32, tag="po")
                                for n in range(NQ):
                                    nc.tensor.matmul(
                                        po[:, :],
                                        aTs[4 * bb + n // 8][:, n % 8, :],
                                        wo_ts[c][:, n, :],
                                        start=(n == 0), stop=(n == NQ - 1))
                                ob = o3pool.tile([128, 256], F32, tag="ob")
                                nc.vector.tensor_copy(ob[:, :], po[:, :])
                                ro = (2 * bb + par) * 128
                                nc.sync.dma_start(
                                    out=out_e[ro:ro + 128, c * 256:(c + 1) * 256],
                                    in_=ob[:, :])

                    oproj(0, a2aA_out)
                    oproj(1, a2aB_out)

    return nc


def _rope_tables(pos_b):
    """pos_b: [3, T] int32 -> sin/cos [T, 64] per mrope."""
    fraction = 2.0 * np.arange(0, H // 2, dtype=np.float64) / H
    timescale = ROPE_THETA ** fraction
    sinusoid = pos_b[:, :, None].astype(np.float64) / timescale  # [3, T, 64]
    freq = sinusoid[0].copy()
    h_idx = np.arange(1, MROPE_SECTION[1] * 3, 3)
    w_idx = np.arange(2, MROPE_SECTION[2] * 3, 3)
    freq[:, h_idx] = sinusoid[1][:, h_idx]
    freq[:, w_idx] = sinusoid[2][:, w_idx]
    return np.sin(freq).astype(np.float32), np.cos(freq).astype(np.float32)


def _tables_for(sin, cos, w):
    """C [T,128], S0 [T,64], S1 [T,64] with norm weight w [128] folded in."""
    C = np.concatenate([cos * w[None, :64], cos * w[None, 64:]], axis=1)
    S0 = sin * w[None, 64:]
    S1 = sin * w[None, :64]
    return (np.ascontiguousarray(C), np.ascontiguousarray(S0), np.ascontiguousarray(S1))


def kernel(x, positions, attn_mask, wq, wk, wv, wo, q_norm_w, k_norm_w):
    x = np.asarray(x, dtype=np.float32)
    positions = np.asarray(positions)
    attn_mask = np.asarray(attn_mask)
    wq = np.asarray(wq, dtype=np.float32)
    wk = np.asarray(wk, dtype=np.float32)
    wv = np.asarray(wv, dtype=np.float32)
    wo = np.asarray(wo, dtype=np.float32)
    q_norm_w = np.asarray(q_norm_w, dtype=np.float32)
    k_norm_w = np.asarray(k_norm_w, dtype=np.float32)

    nc = _build_nc()
    nc.finalize()

    identb = np.eye(128).astype(ml_dtypes.bfloat16)
    wot = wo.transpose(1, 0, 2)  # [hd=128, n=32, d]

    in_maps = []
    for c in range(8):
        b, g = c // 4, c % 4
        xt = np.ascontiguousarray(
            x[b].T.reshape(NDT, 128, NTC, 128).transpose(2, 0, 1, 3)
        ).astype(ml_dtypes.bfloat16)
        kvh = slice(g * 2, g * 2 + 2)
        wq0 = np.ascontiguousarray(
            wq[:, g * 8:g * 8 + 4, :].reshape(D, 512).reshape(NDT, 128, 512)
        ).astype(ml_dtypes.bfloat16)
        wq1 = np.ascontiguousarray(
            wq[:, g * 8 + 4:g * 8 + 8, :].reshape(D, 512).reshape(NDT, 128, 512)
        ).astype(ml_dtypes.bfloat16)
        wkv = np.ascontiguousarray(
            np.concatenate([
                wk[:, kvh, :].reshape(D, 256),
                wv[:, kvh, :].reshape(D, 256),
            ], axis=1).reshape(NDT, 128, 512)
        ).astype(ml_dtypes.bfloat16)
        sin, cos = _rope_tables(np.asarray(positions[:, b, :]))
        cqt, s0qt, s1qt = _tables_for(sin, cos, q_norm_w)
        ckt, s0kt, s1kt = _tables_for(sin, cos, k_norm_w)
        mb = np.empty((NTC, 128, 128), np.float32)
        for i in range(NTC):
            blk = attn_mask[b, i * 128:(i + 1) * 128, i * 128:(i + 1) * 128]
            mb[i] = np.where(blk.T, 0.0, NEG)  # [s, t] orientation
        wob = np.ascontiguousarray(
            wot[:, :, b * DH:(b + 1) * DH].reshape(128, NQ, NWO, 256)
            .transpose(2, 0, 1, 3)).astype(ml_dtypes.bfloat16)
        in_maps.append({
            "xt": xt, "wq0": wq0, "wq1": wq1, "wkv": wkv,
            "cq": cqt, "s0q": s0qt, "s1q": s1qt,
            "ck": ckt, "s0k": s0kt, "s1k": s1kt,
            "maskt": mb, "identb": identb, "wob": wob,
            "onesr": np.ones((1, 128), np.float32),
        })

    res = run_bass_kernel_spmd(nc, in_maps, core_ids=list(range(8)))
    global _LAST
    _LAST = res
    full = np.empty((B, T, D), np.float32)
    for c in range(8):
        b, g = c // 4, c % 4
        r = res.results[c]["out"]  # [512, 1280]
        cols = slice(b * DH, (b + 1) * DH)
        full[0, g * 128:(g + 1) * 128, cols] = r[0:128]
        full[0, (g + 4) * 128:(g + 5) * 128, cols] = r[128:256]
        full[1, g * 128:(g + 1) * 128, cols] = r[256:384]
        full[1, (g + 4) * 128:(g + 5) * 128, cols] = r[384:512]
    return full


# revision 20
# speedup vs baseline: 1.3441x; 1.0145x over previous
"""Distributed Trainium2 kernel for GQA attention block (B=2,T=1024,D=2560,Nq=32,Nkv=8,H=128).

Sharding: 8 cores = 2 batches x 4 head-groups. Core c handles batch c//4 and
q-heads [8g:8g+8), kv-heads [2g:2g+2) where g=c%4. Attention is head-local.
Scores are computed pre-transposed (S.T = kT-block stationary x qT moving) so
AV consumes wide moving operands and needs no per-block PE transposes; softmax
row-sums come from a ones-column matmul, inverted on the scalar engine, and
broadcast to all partitions with a PE outer product.

Two 8-core AllToAlls re-shard heads->tokens: A2A#A carries token chunks 0-3
(fires right after chunks 0-3 are projected + attended, thanks to causality),
A2A#B carries chunks 4-7. Every A2A slot is useful: core (b,g) computes o_proj
columns [1280*b, 1280*b+1280) for BOTH batches' token chunks {g, g+4}, so the
cross-batch shards feed the partner-batch half instead of being discarded.
Output per core: [512, 1280] = (b0 cg, b0 cg+4, b1 cg, b1 cg+4) rows.
"""

import ml_dtypes
import numpy as np

import concourse.bass as bass
import concourse.mybir as mybir
import concourse.tile as tile
from concourse import bacc
from concourse.bass_utils import run_bass_kernel_spmd

F32 = mybir.dt.float32
F32R = mybir.dt.float32r
BF16 = mybir.dt.bfloat16

B, T, D, NQ, NKV, H = 2, 1024, 2560, 32, 8, 128
NDT = D // 128  # 20 contraction tiles
NTC = T // 128  # 8 token chunks
DH = D // 2     # o_proj column half per core
NWO = 5         # o_proj weight sub-tiles of 256 cols
ROPE_THETA = 1000000.0
MROPE_SECTION = (24, 20, 20)
NORM_EPS = 1e-6
SOFT_SCALE = H ** -0.5
NEG = -1e30

EXP_ACT = mybir.ActivationFunctionType.Exp
SQUARE_ACT = mybir.ActivationFunctionType.Square
SQRT_ACT = mybir.ActivationFunctionType.Sqrt
COPY_ACT = mybir.ActivationFunctionType.Copy
MUL_OP = mybir.AluOpType.mult

_LAST = None


def _norm_rope(nc, smpool, tmppool, qrpool, ps, nh, eps_t, ct, s0t, s1t, tagp):
    """RMS-norm over h + rope for nh heads sitting in psum ps[:, :nh*128].

    Returns a BF16 tile [128, nh*128] (token-major) ready for PE transpose.
    The rsqrt scale is fused into the rope products via scalar_tensor_tensor.
    """
    w = nh * 128
    ssq = smpool.tile([128, nh], F32, tag=f"ssq{tagp}")
    for hh in range(nh):
        sq = smpool.tile([128, 128], F32, tag="sq")
        nc.scalar.activation(sq[:, :], ps[:, hh * 128:(hh + 1) * 128], SQUARE_ACT,
                             accum_out=ssq[:, hh:hh + 1])
    srt = smpool.tile([128, nh], F32, tag=f"srt{tagp}")
    nc.scalar.activation(srt[:, :], ssq[:, :], SQRT_ACT, bias=eps_t[:, :], scale=1.0 / H)
    rsq = smpool.tile([128, nh], F32, tag=f"rsq{tagp}")
    nc.vector.reciprocal(rsq[:, :], srt[:, :])
    qm = tmppool.tile([128, w], F32, tag=f"qm{tagp}")
    t1 = tmppool.tile([128, nh * 64], F32, tag=f"t1{tagp}")
    t2 = tmppool.tile([128, nh * 64], F32, tag=f"t2{tagp}")
    qr = qrpool.tile([128, w], BF16, tag=f"qr{tagp}")
    for hh in range(nh):
        rh = rsq[:, hh:hh + 1]
        po = hh * 128
        ho = hh * 64
        # qm = (ps * rsqrt) * (cos*w) ; t1 = (ps_hi * rsqrt) * (sin*w_hi)
        # t2 = (ps_lo * rsqrt) * (sin*w_lo)
        nc.vector.scalar_tensor_tensor(
            qm[:, po:po + 128], ps[:, po:po + 128], rh, ct,
            op0=MUL_OP, op1=MUL_OP)
        nc.vector.scalar_tensor_tensor(
            t1[:, ho:ho + 64], ps[:, po + 64:po + 128], rh, s0t,
            op0=MUL_OP, op1=MUL_OP)
        nc.vector.scalar_tensor_tensor(
            t2[:, ho:ho + 64], ps[:, po:po + 64], rh, s1t,
            op0=MUL_OP, op1=MUL_OP)
    qm4 = qm[:, :].rearrange("p (h x) -> p h x", h=nh)
    qr4 = qr[:, :].rearrange("p (h x) -> p h x", h=nh)
    t14 = t1[:, :].rearrange("p (h x) -> p h x", h=nh)
    t24 = t2[:, :].rearrange("p (h x) -> p h x", h=nh)
    nc.vector.tensor_sub(qr4[:, :, 0:64], qm4[:, :, 0:64], t14)
    nc.vector.tensor_add(qr4[:, :, 64:128], qm4[:, :, 64:128], t24)
    return qr


def _build_nc():
    nc = bacc.Bacc(None, target_bir_lowering=False, num_devices=8)

    xt_e = nc.declare_dram_parameter("xt", [NTC, NDT, 128, 128], BF16, isOutput=False)
    wq0_e = nc.declare_dram_parameter("wq0", [NDT, 128, 512], BF16, isOutput=False)
    wq1_e = nc.declare_dram_parameter("wq1", [NDT, 128, 512], BF16, isOutput=False)
    wkv_e = nc.declare_dram_parameter("wkv", [NDT, 128, 512], BF16, isOutput=False)
    cq_e = nc.declare_dram_parameter("cq", [T, 128], F32, isOutput=False)
    s0q_e = nc.declare_dram_parameter("s0q", [T, 64], F32, isOutput=False)
    s1q_e = nc.declare_dram_parameter("s1q", [T, 64], F32, isOutput=False)
    ck_e = nc.declare_dram_parameter("ck", [T, 128], F32, isOutput=False)
    s0k_e = nc.declare_dram_parameter("s0k", [T, 64], F32, isOutput=False)
    s1k_e = nc.declare_dram_parameter("s1k", [T, 64], F32, isOutput=False)
    maskt_e = nc.declare_dram_parameter("maskt", [NTC, 128, 128], BF16, isOutput=False)
    identb_e = nc.declare_dram_parameter("identb", [128, 128], BF16, isOutput=False)
    wob_e = nc.declare_dram_parameter("wob", [NWO, 128, NQ, 256], BF16, isOutput=False)
    out_e = nc.declare_dram_parameter("out", [512, DH], F32, isOutput=True)

    with tile.TileContext(nc) as tc:
        with (
            tc.tile_pool(name="const", bufs=1) as cpool,
            tc.tile_pool(name="dram", bufs=1, space="DRAM") as dpool,
            tc.tile_pool(name="acts", bufs=1) as apool,
        ):
            eps_t = cpool.tile([128, 1], F32, tag="eps")
            nc.gpsimd.memset(eps_t[:, :], NORM_EPS)
            onesc = cpool.tile([128, 1], BF16, tag="onesc")
            nc.gpsimd.memset(onesc[:, :], 1.0)
            onesrb = cpool.tile([1, 128], BF16, tag="onesrb")
            nc.gpsimd.memset(onesrb[:, :], 1.0)
            identb = cpool.tile([128, 128], BF16, tag="identb")
            nc.sync.dma_start(out=identb[:, :], in_=identb_e[:, :])

            # A2A blocks are [hd, head, t]: contiguous on both DMA sides.
            a2aA_in = dpool.tile([8, 128, 8, 128], BF16, tag="a2aA_in")
            a2aA_out = dpool.tile([8, 128, 8, 128], BF16, tag="a2aA_out")
            a2aB_in = dpool.tile([8, 128, 8, 128], BF16, tag="a2aB_in")
            a2aB_out = dpool.tile([8, 128, 8, 128], BF16, tag="a2aB_out")

            qT = apool.tile([128, 8, T], BF16, tag="qT")       # [h, qhead, t]
            kT = apool.tile([128, 2, T], BF16, tag="kT")       # [h, kvhead, t]
            vN = apool.tile([128, NTC, 256], BF16, tag="vN")   # [s_in_chunk, chunk, kv*128+h]
            maskt = apool.tile([128, NTC, 128], BF16, tag="maskt")

            pend1 = []

            def make_flush(ptpool):
                def flush1():
                    for qr_, tch_, heads_, dest in pend1:
                        for idx, head in enumerate(heads_):
                            pt = ptpool.tile([128, 128], BF16, tag="pt")
                            nc.tensor.transpose(
                                pt[:, :], qr_[:, idx * 128:(idx + 1) * 128], identb[:, :])
                            nc.vector.tensor_copy(
                                dest[:, head, tch_ * 128:(tch_ + 1) * 128], pt[:, :])
                    pend1.clear()
                return flush1

            # ============ attention for one chunk-group ============
            # Group = 4 query chunks [c0, c0+4). Heads are processed in
            # pairs sharing a kv head: scores for both heads land in one
            # [128, 1024] psum tile (each half within a psum bank), the
            # mask add + exp run as single strided ops over both heads.
            # AV + ones-rowsum accumulate per head with one matmul per
            # s-block (stop is sim-only; skip_group_check bypasses the
            # zero-region tracker for the interleaved accumulation).
            def attn_group(c0, otg, pools):
                stpool, ovpool, rspool, rbpool, epool, scpool = pools
                jmax = c0 + 4
                pend_epi = []
                pend_av = []

                def flush_epi():
                    for hq_, ov_, rs_ in pend_epi:
                        # rowsum -> sbuf bf16, broadcast via PE outer product,
                        # fast reciprocal across all 128 partitions.
                        rss = scpool.tile([1, 512], BF16, tag="rss")
                        nc.scalar.activation(rss[:, :], rs_[:, :], COPY_ACT)
                        sumb = rbpool.tile([128, 512], F32, tag="sumb")
                        nc.tensor.matmul(sumb[:, :], onesrb[:, :], rss[:, :],
                                         start=True, stop=True)
                        rcps = scpool.tile([128, 512], F32, tag="rcps")
                        nc.vector.reciprocal_approx_fast(rcps[:, :], sumb[:, :])
                        ovv = ov_[:, :].rearrange("p (c t) -> p c t", c=4)
                        rbv = rcps[:, :].rearrange("p (c t) -> p c t", c=4)
                        nc.vector.tensor_mul(otg[:, :, hq_, :], ovv, rbv)
                    pend_epi.clear()

                def flush_av():
                    j_, w_, est2_, ovs_, rss_, kv_ = pend_av.pop(0)
                    lo_ = max(j_, c0)
                    co = (lo_ - c0) * 128
                    vt = vN[:, j_, kv_ * 128:(kv_ + 1) * 128]
                    st_ = (j_ == 0)
                    for i_ in range(2):
                        mv = est2_[:, i_ * 512:i_ * 512 + w_]
                        nc.tensor.matmul(ovs_[i_][:, co:512], vt, mv,
                                         start=st_, stop=True, skip_group_check=True)
                        nc.tensor.matmul(rss_[i_][:, co:512], onesc[:, :], mv,
                                         start=st_, stop=True, skip_group_check=True)

                for hq in range(0, 8, 2):
                    kv = hq // 4
                    ov0 = ovpool.tile([128, 512], F32, tag="ov0")
                    ov1 = ovpool.tile([128, 512], F32, tag="ov1")
                    rs0 = rspool.tile([1, 512], F32, tag="rs0")
                    rs1 = rspool.tile([1, 512], F32, tag="rs1")
                    for j in range(jmax):
                        lo_ = max(j, c0)
                        w = (c0 + 4 - lo_) * 128
                        st2 = stpool.tile([128, 1024], F32, tag="st2")
                        for i_, h_ in enumerate((hq, hq + 1)):
                            nc.tensor.matmul(
                                st2[:, i_ * 512:i_ * 512 + w],
                                kT[:, kv, j * 128:(j + 1) * 128],
                                qT[:, h_, lo_ * 128:(c0 + 4) * 128],
                                start=True, stop=True,
                            )
                        sview = st2[:, :].rearrange("p (i x) -> p i x", i=2)
                        est2 = epool.tile([128, 1024], BF16, tag="est2")
                        e2v = est2[:, :].rearrange("p (i x) -> p i x", i=2)
                        if w == 512:
                            nc.scalar.activation(est2[:, 0:1024], st2[:, 0:1024],
                                                 EXP_ACT, scale=SOFT_SCALE)
                        else:
                            nc.scalar.activation(e2v[:, :, 0:w], sview[:, :, 0:w],
                                                 EXP_ACT, scale=SOFT_SCALE)
                        if j >= c0:
                            # binary causal mask applied post-exp in sbuf
                            mb = maskt[:, j, :].unsqueeze(1).broadcast_to([128, 2, 128])
                            nc.vector.tensor_mul(e2v[:, 0:2, 0:128], e2v[:, 0:2, 0:128], mb)
                        while len(pend_av) >= 2:
                            flush_av()
                        if hq > 0 and j == 1:
                            flush_epi()
                        pend_av.append((j, w, est2, (ov0, ov1), (rs0, rs1), kv))
                    pend_epi.append((hq, ov0, rs0))
                    pend_epi.append((hq + 1, ov1, rs1))
                while pend_av:
                    flush_av()
                flush_epi()

            # ================= Phase 1 + attention =================
            with (
                tc.tile_pool(name="p1tab", bufs=1) as tabpool,
                tc.tile_pool(name="p1x", bufs=3) as xpool,
                tc.tile_pool(name="p1w", bufs=60) as wpool,
                tc.tile_pool(name="p1qr", bufs=2) as qrpool,
                tc.tile_pool(name="p1tmp", bufs=1) as tmppool,
                tc.tile_pool(name="p1sm", bufs=2) as smpool,
            ):
                # sync-queue DMA priority order: wkv, x0, wq0, x1,
                # rope tables, wq1, x2..x7 — chunk 0 computes while its
                # later weight groups stream in.
                xcs = {}

                def xload(tch):
                    xc = xpool.tile([128, NDT, 128], BF16, tag="xc")
                    nc.sync.dma_start(
                        out=xc[:, :, :],
                        in_=xt_e[tch, :, :, :].rearrange("d p t -> p d t"))
                    xcs[tch] = xc

                def wload(src_e, n=NDT):
                    ts = []
                    for d in range(n):
                        wt = wpool.tile([128, 512], BF16, tag="wt")
                        nc.sync.dma_start(out=wt[:, :], in_=src_e[d, :, :])
                        ts.append(wt)
                    return ts

                def ld(name, shp, src, rearr):
                    t = tabpool.tile(shp, F32, tag=name)
                    nc.sync.dma_start(out=t[:, :, :], in_=src[:, :].rearrange(rearr, p=128))
                    return t

                wkvts = wload(wkv_e)
                xload(0)
                wq0ts = wload(wq0_e)
                xload(1)
                cq = ld("cq", [128, NTC, 128], cq_e, "(c p) m -> p c m")
                s0q = ld("s0q", [128, NTC, 64], s0q_e, "(c p) m -> p c m")
                s1q = ld("s1q", [128, NTC, 64], s1q_e, "(c p) m -> p c m")
                ck = ld("ck", [128, NTC, 128], ck_e, "(c p) m -> p c m")
                s0k = ld("s0k", [128, NTC, 64], s0k_e, "(c p) m -> p c m")
                s1k = ld("s1k", [128, NTC, 64], s1k_e, "(c p) m -> p c m")
                nc.sync.dma_start(out=maskt[:, :, :],
                                  in_=maskt_e[:, :, :].rearrange("i p m -> p i m"))
                wq1ts = wload(wq1_e)
                for tch in range(2, NTC):
                    xload(tch)

                def proj_chunks(chunks, pspool, flush1):
                    for tch in chunks:
                        xc = xcs.pop(tch)
                        pskv = pspool.tile([128, 512], F32, tag="pskv")
                        for d in range(NDT):
                            nc.tensor.matmul(pskv[:, :], xc[:, d, :], wkvts[d][:, :],
                                             start=(d == 0), stop=(d == NDT - 1))
                        ps0 = pspool.tile([128, 512], F32, tag="ps0")
                        for d in range(NDT):
                            nc.tensor.matmul(ps0[:, :], xc[:, d, :], wq0ts[d][:, :],
                                             start=(d == 0), stop=(d == NDT - 1))
                        ps1 = pspool.tile([128, 512], F32, tag="ps1")
                        for d in range(NDT):
                            nc.tensor.matmul(ps1[:, :], xc[:, d, :], wq1ts[d][:, :],
                                             start=(d == 0), stop=(d == NDT - 1))
                        flush1()
                        kr = _norm_rope(nc, smpool, tmppool, qrpool, pskv[:, 0:256], 2,
                                        eps_t, ck[:, tch, :], s0k[:, tch, :], s1k[:, tch, :], "k")
                        pend1.append((kr, tch, [0, 1], kT))
                        nc.vector.tensor_copy(vN[:, tch, :], pskv[:, 256:512])
                        qr0 = _norm_rope(nc, smpool, tmppool, qrpool, ps0[:, :], 4,
                                         eps_t, cq[:, tch, :], s0q[:, tch, :], s1q[:, tch, :], "q0")
                        pend1.append((qr0, tch, [0, 1, 2, 3], qT))
                        qr1 = _norm_rope(nc, smpool, tmppool, qrpool, ps1[:, :], 4,
                                         eps_t, cq[:, tch, :], s0q[:, tch, :], s1q[:, tch, :], "q1")
                        pend1.append((qr1, tch, [4, 5, 6, 7], qT))

                with (
                    tc.tile_pool(name="ppA", bufs=2, space="PSUM") as pspoolA,
                    tc.tile_pool(name="ptA", bufs=2, space="PSUM") as ptpoolA,
                ):
                    fl = make_flush(ptpoolA)
                    proj_chunks([0, 1, 2, 3], pspoolA, fl)
                    fl()

                with (
                    tc.tile_pool(name="stA", bufs=1, space="PSUM") as stp,
                    tc.tile_pool(name="ovA", bufs=1, space="PSUM") as ovp,
                    tc.tile_pool(name="rsA", bufs=1, space="PSUM") as rsp,
                    tc.tile_pool(name="rbA", bufs=1, space="PSUM") as rbp,
                    tc.tile_pool(name="eA", bufs=3) as ep,
                    tc.tile_pool(name="scA", bufs=2) as scp,
                    tc.tile_pool(name="otA", bufs=1) as otp,
                ):
                    otgA = otp.tile([128, 4, 8, 128], BF16, tag="otgA")
                    attn_group(0, otgA, (stp, ovp, rsp, rbp, ep, scp))
                    for s in range(8):
                        nc.sync.dma_start(out=a2aA_in[s, :, :, :], in_=otgA[:, s % 4, :, :])
                    nc.gpsimd.collective_compute(
                        "AllToAll", mybir.AluOpType.bypass,
                        replica_groups=[[0, 1, 2, 3, 4, 5, 6, 7]],
                        ins=[a2aA_in[:, :, :, :].opt()],
                        outs=[a2aA_out[:, :, :, :].opt()],
                    )

                with (
                    tc.tile_pool(name="ppB", bufs=2, space="PSUM") as pspoolB,
                    tc.tile_pool(name="ptB", bufs=2, space="PSUM") as ptpoolB,
                ):
                    fl = make_flush(ptpoolB)
                    proj_chunks([4, 5, 6, 7], pspoolB, fl)
                    fl()

            # proj sbuf pools closed; o_proj weights load into freed space
            with (
                tc.tile_pool(name="wo3", bufs=NWO) as wopool,
                tc.tile_pool(name="p3a", bufs=1) as a3pool,
                tc.tile_pool(name="p3o", bufs=3) as o3pool,
            ):
                wo_ts = []
                for c in range(NWO):
                    wo_t = wopool.tile([128, NQ, 256], BF16, tag="wo_t")
                    nc.gpsimd.dma_start(out=wo_t[:, :, :], in_=wob_e[c, :, :, :])
                    wo_ts.append(wo_t)

                with (
                    tc.tile_pool(name="stB", bufs=1, space="PSUM") as stp,
                    tc.tile_pool(name="ovB", bufs=1, space="PSUM") as ovp,
                    tc.tile_pool(name="rsB", bufs=1, space="PSUM") as rsp,
                    tc.tile_pool(name="rbB", bufs=1, space="PSUM") as rbp,
                    tc.tile_pool(name="eB", bufs=3) as ep,
                    tc.tile_pool(name="scB", bufs=2) as scp,
                    tc.tile_pool(name="otB", bufs=1) as otp,
                ):
                    otgB = otp.tile([128, 4, 8, 128], BF16, tag="otgB")
                    attn_group(4, otgB, (stp, ovp, rsp, rbp, ep, scp))
                    for s in range(8):
                        nc.sync.dma_start(out=a2aB_in[s, :, :, :], in_=otgB[:, s % 4, :, :])
                    nc.gpsimd.collective_compute(
                        "AllToAll", mybir.AluOpType.bypass,
                        replica_groups=[[0, 1, 2, 3, 4, 5, 6, 7]],
                        ins=[a2aB_in[:, :, :, :].opt()],
                        outs=[a2aB_out[:, :, :, :].opt()],
                    )

                # ============ Phase 3: o_proj (both batches, my D-half) ====
                with tc.tile_pool(name="poP", bufs=2, space="PSUM") as popool:
                    def oproj(par, a2a_out_t):
                        # aT[bb][g'] = heads 8g'..8g'+8 of batch bb, my chunk
                        aTs = []
                        for s in range(8):
                            aT = a3pool.tile([128, 8, 128], BF16, tag=f"aT{par}{s}")
                            nc.gpsimd.dma_start(out=aT[:, :, :], in_=a2a_out_t[s, :, :, :])
                            aTs.append(aT)
                        for bb in range(2):
                            for c in range(NWO):
                                po = popool.tile([128, 256], F32, tag="po")
                                for n in range(NQ):
                                    nc.tensor.matmul(
                                        po[:, :],
                                        aTs[4 * bb + n // 8][:, n % 8, :],
                                        wo_ts[c][:, n, :],
                                        start=(n == 0), stop=(n == NQ - 1))
                                ob = o3pool.tile([128, 256], F32, tag="ob")
                                nc.vector.tensor_copy(ob[:, :], po[:, :])
                                ro = (2 * bb + par) * 128
                                nc.sync.dma_start(
                                    out=out_e[ro:ro + 128, c * 256:(c + 1) * 256],
                                    in_=ob[:, :])

                    oproj(0, a2aA_out)
                    oproj(1, a2aB_out)

    return nc


def _rope_tables(pos_b):
    """pos_b: [3, T] int32 -> sin/cos [T, 64] per mrope."""
    fraction = 2.0 * np.arange(0, H // 2, dtype=np.float64) / H
    timescale = ROPE_THETA ** fraction
    sinusoid = pos_b[:, :, None].astype(np.float64) / timescale  # [3, T, 64]
    freq = sinusoid[0].copy()
    h_idx = np.arange(1, MROPE_SECTION[1] * 3, 3)
    w_idx = np.arange(2, MROPE_SECTION[2] * 3, 3)
    freq[:, h_idx] = sinusoid[1][:, h_idx]
    freq[:, w_idx] = sinusoid[2][:, w_idx]
    return np.sin(freq).astype(np.float32), np.cos(freq).astype(np.float32)


def _tables_for(sin, cos, w):
    """C [T,128], S0 [T,64], S1 [T,64] with norm weight w [128] folded in."""
    C = np.concatenate([cos * w[None, :64], cos * w[None, 64:]], axis=1)
    S0 = sin * w[None, 64:]
    S1 = sin * w[None, :64]
    return (np.ascontiguousarray(C), np.ascontiguousarray(S0), np.ascontiguousarray(S1))


def kernel(x, positions, attn_mask, wq, wk, wv, wo, q_norm_w, k_norm_w):
    x = np.asarray(x, dtype=np.float32)
    positions = np.asarray(positions)
    attn_mask = np.asarray(attn_mask)
    wq = np.asarray(wq, dtype=np.float32)
    wk = np.asarray(wk, dtype=np.float32)
    wv = np.asarray(wv, dtype=np.float32)
    wo = np.asarray(wo, dtype=np.float32)
    q_norm_w = np.asarray(q_norm_w, dtype=np.float32)
    k_norm_w = np.asarray(k_norm_w, dtype=np.float32)

    nc = _build_nc()
    nc.finalize()

    identb = np.eye(128).astype(ml_dtypes.bfloat16)
    wot = wo.transpose(1, 0, 2)  # [hd=128, n=32, d]

    in_maps = []
    for c in range(8):
        b, g = c // 4, c % 4
        xt = np.ascontiguousarray(
            x[b].T.reshape(NDT, 128, NTC, 128).transpose(2, 0, 1, 3)
        ).astype(ml_dtypes.bfloat16)
        kvh = slice(g * 2, g * 2 + 2)
        wq0 = np.ascontiguousarray(
            wq[:, g * 8:g * 8 + 4, :].reshape(D, 512).reshape(NDT, 128, 512)
        ).astype(ml_dtypes.bfloat16)
        wq1 = np.ascontiguousarray(
            wq[:, g * 8 + 4:g * 8 + 8, :].reshape(D, 512).reshape(NDT, 128, 512)
        ).astype(ml_dtypes.bfloat16)
        wkv = np.ascontiguousarray(
            np.concatenate([
                wk[:, kvh, :].reshape(D, 256),
                wv[:, kvh, :].reshape(D, 256),
            ], axis=1).reshape(NDT, 128, 512)
        ).astype(ml_dtypes.bfloat16)
        sin, cos = _rope_tables(np.asarray(positions[:, b, :]))
        cqt, s0qt, s1qt = _tables_for(sin, cos, q_norm_w)
        ckt, s0kt, s1kt = _tables_for(sin, cos, k_norm_w)
        mb = np.empty((NTC, 128, 128), np.float32)
        for i in range(NTC):
            blk = attn_mask[b, i * 128:(i + 1) * 128, i * 128:(i + 1) * 128]
            mb[i] = np.where(blk.T, 1.0, 0.0)  # [s, t], post-exp binary mask
        mb = mb.astype(ml_dtypes.bfloat16)
        wob = np.ascontiguousarray(
            wot[:, :, b * DH:(b + 1) * DH].reshape(128, NQ, NWO, 256)
            .transpose(2, 0, 1, 3)).astype(ml_dtypes.bfloat16)
        in_maps.append({
            "xt": xt, "wq0": wq0, "wq1": wq1, "wkv": wkv,
            "cq": cqt, "s0q": s0qt, "s1q": s1qt,
            "ck": ckt, "s0k": s0kt, "s1k": s1kt,
            "maskt": mb, "identb": identb, "wob": wob,
        })

    res = run_bass_kernel_spmd(nc, in_maps, core_ids=list(range(8)))
    global _LAST
    _LAST = res
    full = np.empty((B, T, D), np.float32)
    for c in range(8):
        b, g = c // 4, c % 4
        r = res.results[c]["out"]  # [512, 1280]
        cols = slice(b * DH, (b + 1) * DH)
        full[0, g * 128:(g + 1) * 128, cols] = r[0:128]
        full[0, (g + 4) * 128:(g + 5) * 128, cols] = r[128:256]
        full[1, g * 128:(g + 1) * 128, cols] = r[256:384]
        full[1, (g + 4) * 128:(g + 5) * 128, cols] = r[384:512]
    return full
